# revision 1
# baseline (speedup 1.0000x reference)
"""MLA (DeepSeek-style) attention block on 8 Trainium2 NeuronCores.

Sharding:
  phase 1 (token-parallel, 8 x 512 tokens): LoRA-A down-projections + rmsnorm
    + k_pe rope; small AllGather of the kv latents (576 dims); q up-projection
    for ALL 16 heads on the token side + rope, shipped to head shards via two
    shard-aligned AllToAlls (pe+even-nope first, odd-nope second).
  phase 2 (head-parallel, 2 heads x 2 batches per core): k/v up-projection
    from gathered kv latents; causal flash attention (k-major scores, exp on
    ScalarE, ones-matmul denominator, reciprocal + K=1 broadcast matmul for
    the divide).
  output: two AllToAlls (even heads overlap the odd-head attention; odd heads
    overlap the even half of the token-parallel output projection).

bf16 matmuls, fp32 PSUM accumulation + softmax statistics, fp32 output.
"""
import sys
from contextlib import ExitStack

sys.path.insert(0, "/opt/trn_rl_repo")

import numpy as np
import ml_dtypes

import concourse.bacc as bacc
import concourse.mybir as mybir
import concourse.tile as tile
from concourse.bass_utils import run_bass_kernel_spmd

# ---- problem sizes (hardcoded per spec) ----
HID = 2048; H = 16; QLR = 1536; KVLR = 512
DN = 128; DR = 64; DV = 128; DQ = DN + DR
B = 2; S = 2048
THETA = 10000.0; EPS = 1e-6

NCORES = 8
T = B * S              # 4096 flattened tokens
TPC = T // NCORES      # 512 tokens per core
HPC = H // NCORES      # 2 heads per core
P = 128
NHID = HID // P        # 16
NQLR = QLR // P        # 12
CKW = KVLR + DR        # 576
QT_PER_B = S // 512    # 4 q-tiles of 512 per (b,h) unit
KB_PER_B = S // P      # 16 k-chunks of 128 per batch
WKK = HPC * DN         # 256

BF16 = mybir.dt.bfloat16
F32 = mybir.dt.float32
AF = mybir.ActivationFunctionType

_NC_CACHE = None


def _rope_dual(nc, pool, out_bf16, ps, cos_sb, sin_sb, tag):
    """RoPE on a [128, W] psum holding two 64-row head groups; writes bf16."""
    W = 512
    HDR = DR // 2
    rot = pool.tile([P, W], F32, tag=f"{tag}rot", name=f"{tag}rot")
    for g in range(2):
        o = g * DR
        nc.scalar.mul(rot[o:o + HDR, :], ps[o + HDR:o + DR, :], -1.0)
        nc.scalar.copy(rot[o + HDR:o + DR, :], ps[o:o + HDR, :])
    t1 = pool.tile([P, W], F32, tag=f"{tag}t1", name=f"{tag}t1")
    nc.vector.tensor_mul(t1[:], ps[:], cos_sb[:])
    nc.vector.tensor_mul(rot[:], rot[:], sin_sb[:])
    nc.vector.tensor_add(out_bf16[:], t1[:], rot[:])


def _phase1ab(nc, tc, ps1, ps1s, ps1b, hidT, wqaT, wkvaT, latkv_in, latkv_all,
              cqn_sb, ones_col, ones_row, eps_t, cos_sb, sin_sb, RG):
    """ckv path (+ kv AllGather) then cq path; SBUF freed on exit."""
    with tc.tile_pool(name="p1a", bufs=1) as p1a, \
         tc.tile_pool(name="p1t", bufs=2) as p1t, \
         tc.tile_pool(name="p1n", bufs=1) as p1n:
        hid_ch = [p1a.tile([P, TPC], BF16, tag=f"hid{kc}", name=f"hid{kc}")
                  for kc in range(NHID)]
        wkva_ch = [p1a.tile([P, CKW], BF16, tag=f"wkva{kc}", name=f"wkva{kc}")
                   for kc in range(NHID)]
        wqa_ch = [p1a.tile([P, QLR], BF16, tag=f"wqa{kc}", name=f"wqa{kc}")
                  for kc in range(NHID)]
        for kc in range(NHID):
            nc.sync.dma_start(hid_ch[kc][:], hidT.ap()[kc * P:(kc + 1) * P, :])
            nc.sync.dma_start(wkva_ch[kc][:], wkvaT.ap()[kc * P:(kc + 1) * P, :])
        for kc in range(NHID):
            nc.sync.dma_start(wqa_ch[kc][:], wqaT.ap()[kc * P:(kc + 1) * P, :])

        # --- ckv joint (d-major): 4 normed blocks + k_pe block ---
        with tc.tile_pool(name="p1ckv", bufs=1) as p1ckv:
            ckv_f32 = p1ckv.tile([P, 4 * TPC], BF16)
            ssq_kv = ps1s.tile([1, TPC], F32)
            for m in range(4):
                ps = ps1.tile([P, TPC], F32, tag="proj")
                for kc in range(NHID):
                    nc.tensor.matmul(ps[:], wkva_ch[kc][:, m * P:(m + 1) * P],
                                     hid_ch[kc][:],
                                     start=(kc == 0), stop=(kc == NHID - 1))
                nc.scalar.copy(ckv_f32[:, m * TPC:(m + 1) * TPC], ps[:])
                sq = p1t.tile([P, TPC], BF16, tag="sq")
                nc.vector.tensor_mul(sq[:], ckv_f32[:, m * TPC:(m + 1) * TPC],
                                     ckv_f32[:, m * TPC:(m + 1) * TPC])
                nc.tensor.matmul(ssq_kv[:], ones_col[:], sq[:],
                                 start=(m == 0), stop=(m == 3),
                                 skip_group_check=True)

            # k_pe block [64, TPC] + rope (shared across heads)
            ps_pe = ps1.tile([DR, TPC], F32, tag="proj")
            for kc in range(NHID):
                nc.tensor.matmul(ps_pe[:], wkva_ch[kc][:, KVLR:CKW],
                                 hid_ch[kc][:],
                                 start=(kc == 0), stop=(kc == NHID - 1))
            HDR = DR // 2
            rot = p1t.tile([DR, TPC], F32, tag="rot")
            nc.scalar.mul(rot[0:HDR, :], ps_pe[HDR:DR, :], -1.0)
            nc.scalar.copy(rot[HDR:DR, :], ps_pe[0:HDR, :])
            t1 = p1t.tile([DR, TPC], F32, tag="t1")
            nc.vector.tensor_mul(t1[:], ps_pe[:], cos_sb[0:DR, :])
            nc.vector.tensor_mul(rot[:], rot[:], sin_sb[0:DR, :])
            pe_out = p1t.tile([DR, TPC], BF16, tag="peo")
            nc.vector.tensor_add(pe_out[:], t1[:], rot[:])
            nc.sync.dma_start(latkv_in[KVLR:CKW, :], pe_out[:])

            kv_norm = p1n.tile([1, TPC], F32, tag="nrm")
            nc.scalar.activation(kv_norm[:], ssq_kv[:], AF.Sqrt, bias=eps_t[:],
                                 scale=1.0 / KVLR)
            rn_kv = p1n.tile([1, TPC], F32, tag="rn")
            nc.vector.reciprocal(rn_kv[:], kv_norm[:])
            bkv = ps1b.tile([P, TPC], F32, tag="bc")
            nc.tensor.matmul(bkv[:], ones_row[:], rn_kv[:], start=True, stop=True)
            for m in range(4):
                lat_sb = p1t.tile([P, TPC], BF16, tag="sq")
                nc.vector.tensor_mul(lat_sb[:], ckv_f32[:, m * TPC:(m + 1) * TPC], bkv[:])
                nc.sync.dma_start(latkv_in[m * P:(m + 1) * P, :], lat_sb[:])

            nc.gpsimd.collective_compute(
                "AllGather", mybir.AluOpType.bypass, replica_groups=RG,
                ins=[latkv_in.opt()], outs=[latkv_all.opt()])

        # --- cq (bf16 storage) + rmsnorm ---
        cq_bf = p1a.tile([P, NQLR * TPC], BF16)
        ssq_q = ps1s.tile([1, TPC], F32)
        for m in range(NQLR):
            ps = ps1.tile([P, TPC], F32, tag="proj")
            for kc in range(NHID):
                nc.tensor.matmul(ps[:], wqa_ch[kc][:, m * P:(m + 1) * P],
                                 hid_ch[kc][:],
                                 start=(kc == 0), stop=(kc == NHID - 1))
            nc.scalar.copy(cq_bf[:, m * TPC:(m + 1) * TPC], ps[:])
            sq = p1t.tile([P, TPC], BF16, tag="sq")
            nc.vector.tensor_mul(sq[:], cq_bf[:, m * TPC:(m + 1) * TPC],
                                 cq_bf[:, m * TPC:(m + 1) * TPC])
            nc.tensor.matmul(ssq_q[:], ones_col[:], sq[:],
                             start=(m == 0), stop=(m == NQLR - 1),
                             skip_group_check=True)
        sq_norm = p1n.tile([1, TPC], F32, tag="nrm")
        nc.scalar.activation(sq_norm[:], ssq_q[:], AF.Sqrt, bias=eps_t[:],
                             scale=1.0 / QLR)
        rn_q = p1n.tile([1, TPC], F32, tag="rn")
        nc.vector.reciprocal(rn_q[:], sq_norm[:])
        bq = ps1b.tile([P, TPC], F32, tag="bc")
        nc.tensor.matmul(bq[:], ones_row[:], rn_q[:], start=True, stop=True)
        for m in range(NQLR):
            nc.vector.tensor_mul(cqn_sb[:, m * TPC:(m + 1) * TPC],
                                 cq_bf[:, m * TPC:(m + 1) * TPC], bq[:])


def build_nc():
    nc = bacc.Bacc(None, target_bir_lowering=False, debug=False, num_devices=NCORES)

    # ---- per-core external inputs ----
    hidT = nc.dram_tensor("hidT", [HID, TPC], BF16, kind="ExternalInput")
    wqaT = nc.dram_tensor("wqaT", [HID, QLR], BF16, kind="ExternalInput")
    wkvaT = nc.dram_tensor("wkvaT", [HID, CKW], BF16, kind="ExternalInput")
    wqbT = nc.dram_tensor("wqbT", [QLR, H * DQ], BF16, kind="ExternalInput")
    wkvbkT = nc.dram_tensor("wkvbkT", [KVLR, HPC * DN], BF16, kind="ExternalInput")
    wkvbvT = nc.dram_tensor("wkvbvT", [KVLR, HPC * DV], BF16, kind="ExternalInput")
    woT = nc.dram_tensor("woT", [H * DV, HID], BF16, kind="ExternalInput")
    cosd = nc.dram_tensor("cosd", [P, TPC], F32, kind="ExternalInput")
    sind = nc.dram_tensor("sind", [P, TPC], F32, kind="ExternalInput")
    masks = nc.dram_tensor("masks", [P, 4 * 512], BF16, kind="ExternalInput")
    outT = nc.dram_tensor("outT", [HID, TPC], F32, kind="ExternalOutput")

    RG = [list(range(NCORES))]

    with tile.TileContext(nc) as tc:
        with tc.tile_pool(name="dram", bufs=1, space="DRAM") as dram, \
             tc.tile_pool(name="const", bufs=1) as const:
            latkv_in = dram.tile([CKW, TPC], BF16)
            latkv_all = dram.tile([NCORES * CKW, TPC], BF16, addr_space="Shared")
            qa_in = dram.tile([NCORES * 2 * P, TPC], BF16)   # [pe2|h0n] per pair
            qa_out = dram.tile([NCORES * 2 * P, TPC], BF16)
            qb_in = dram.tile([NCORES * P, TPC], BF16)       # h1n per pair
            qb_out = dram.tile([NCORES * P, TPC], BF16)
            oa_in = dram.tile([NCORES * DV, TPC], BF16)      # even heads out
            oa_out = dram.tile([NCORES * DV, TPC], BF16)
            ob_in = dram.tile([NCORES * DV, TPC], BF16)      # odd heads out
            ob_out = dram.tile([NCORES * DV, TPC], BF16)

            ones_col = const.tile([P, 1], BF16)
            nc.vector.memset(ones_col[:], 1.0)
            ones_row = const.tile([1, P], F32)
            nc.vector.memset(ones_row[:], 1.0)
            eps_t = const.tile([1, 1], F32)
            nc.vector.memset(eps_t[:], EPS)
            cos_sb = const.tile([P, TPC], F32)
            sin_sb = const.tile([P, TPC], F32)
            nc.sync.dma_start(cos_sb[:], cosd.ap()[:])
            nc.sync.dma_start(sin_sb[:], sind.ap()[:])

            # ============ Phase 1: wqb prefetch + token-parallel compute ============
            p1q_stack = ExitStack()
            p1q = p1q_stack.enter_context(tc.tile_pool(name="p1q", bufs=1))
            ps1_stack = ExitStack()
            ps1 = ps1_stack.enter_context(tc.tile_pool(name="ps1", bufs=3, space="PSUM"))
            ps1s = ps1_stack.enter_context(tc.tile_pool(name="ps1s", bufs=1, space="PSUM"))
            ps1b = ps1_stack.enter_context(tc.tile_pool(name="ps1b", bufs=2, space="PSUM"))
            if True:
                WQB = H * DQ  # 3072
                wqb_ch = [p1q.tile([P, WQB], BF16, tag=f"wqb{kc}", name=f"wqb{kc}")
                          for kc in range(NQLR)]
                for kc in range(NQLR):
                    nc.sync.dma_start(wqb_ch[kc][:], wqbT.ap()[kc * P:(kc + 1) * P, :])
                cqn_sb = p1q.tile([P, NQLR * TPC], BF16)
                # ---- phase 1a/1b scope (freed before q up-projection) ----
                _phase1ab(nc, tc, ps1, ps1s, ps1b, hidT, wqaT, wkvaT,
                          latkv_in, latkv_all, cqn_sb, ones_col, ones_row,
                          eps_t, cos_sb, sin_sb, RG)
                # ============ Phase 1c: q up-projection for ALL heads ============
                with tc.tile_pool(name="p1qt", bufs=3) as p1qt:
                    for mb in range(16):
                        ps = ps1.tile([P, TPC], F32, tag="proj")
                        for kc in range(NQLR):
                            nc.tensor.matmul(ps[:], wqb_ch[kc][:, mb * P:(mb + 1) * P],
                                             cqn_sb[:, kc * TPC:(kc + 1) * TPC],
                                             start=(kc == 0), stop=(kc == NQLR - 1))
                        qo = p1qt.tile([P, TPC], BF16, tag="qo")
                        if mb % 2 == 0:  # pe2 block -> rope
                            _rope_dual(nc, p1qt, qo, ps, cos_sb, sin_sb, "q")
                        else:
                            nc.scalar.copy(qo[:], ps[:])
                        nc.sync.dma_start(qa_in[mb * P:(mb + 1) * P, :], qo[:])
                    nc.gpsimd.collective_compute(
                        "AllToAll", mybir.AluOpType.bypass, replica_groups=RG,
                        ins=[qa_in.opt()], outs=[qa_out.opt()])
                    for mb in range(8):
                        ps = ps1.tile([P, TPC], F32, tag="proj")
                        for kc in range(NQLR):
                            nc.tensor.matmul(ps[:], wqb_ch[kc][:, (16 + mb) * P:(17 + mb) * P],
                                             cqn_sb[:, kc * TPC:(kc + 1) * TPC],
                                             start=(kc == 0), stop=(kc == NQLR - 1))
                        qo = p1qt.tile([P, TPC], BF16, tag="qo")
                        nc.scalar.copy(qo[:], ps[:])
                        nc.sync.dma_start(qb_in[mb * P:(mb + 1) * P, :], qo[:])
                    nc.gpsimd.collective_compute(
                        "AllToAll", mybir.AluOpType.bypass, replica_groups=RG,
                        ins=[qb_in.opt()], outs=[qb_out.opt()])
            ps1_stack.close()

            # ===== Phase 2: k/v up-proj + q receive (overlaps the q AllToAlls) =====
            with tc.tile_pool(name="att_a", bufs=1) as att_a:
                knope = att_a.tile([P, 2 * T], BF16)
                kpe2 = att_a.tile([P, T], BF16)    # k_pe duplicated rows
                v_sb = att_a.tile([P, (T // P) * WKK], BF16)
                qnope = att_a.tile([P, 2 * T], BF16)
                qpe = att_a.tile([P, T], BF16)     # rows 0-63 h0, 64-127 h1

                with tc.tile_pool(name="p2w", bufs=1) as p2w, \
                     tc.tile_pool(name="p2a", bufs=3) as p2a, \
                     tc.tile_pool(name="ps2", bufs=4, space="PSUM") as ps2:
                    wkk_sb = p2w.tile([P, 4 * WKK], BF16)
                    wkv_sb = p2w.tile([P, 4 * WKK], BF16)
                    for kc in range(4):
                        nc.sync.dma_start(wkk_sb[:, kc * WKK:(kc + 1) * WKK],
                                          wkvbkT.ap()[kc * P:(kc + 1) * P, :])
                        nc.sync.dma_start(wkv_sb[:, kc * WKK:(kc + 1) * WKK],
                                          wkvbvT.ap()[kc * P:(kc + 1) * P, :])
                    for j in range(NCORES):
                        basek = j * CKW
                        ckv_j = p2a.tile([P, 4 * TPC], BF16, tag="ckvj")
                        for r in range(4):
                            nc.sync.dma_start(ckv_j[:, r * TPC:(r + 1) * TPC],
                                              latkv_all[basek + r * P: basek + (r + 1) * P, :])
                        nc.sync.dma_start(kpe2[0:DR, j * TPC:(j + 1) * TPC],
                                          latkv_all[basek + KVLR: basek + CKW, :])
                        nc.sync.dma_start(kpe2[DR:P, j * TPC:(j + 1) * TPC],
                                          latkv_all[basek + KVLR: basek + CKW, :])
                        for m in range(HPC):
                            ps = ps2.tile([P, TPC], F32, tag="proj")
                            for kc in range(4):
                                nc.tensor.matmul(
                                    ps[:], wkk_sb[:, kc * WKK + m * P: kc * WKK + (m + 1) * P],
                                    ckv_j[:, kc * TPC:(kc + 1) * TPC],
                                    start=(kc == 0), stop=(kc == 3))
                            nc.scalar.copy(knope[:, m * T + j * TPC: m * T + (j + 1) * TPC], ps[:])
                        for tb in range(TPC // P):
                            ps = ps2.tile([P, WKK], F32, tag="proj")
                            for kc in range(4):
                                nc.tensor.matmul(
                                    ps[:], ckv_j[:, kc * TPC + tb * P: kc * TPC + (tb + 1) * P],
                                    wkv_sb[:, kc * WKK:(kc + 1) * WKK],
                                    start=(kc == 0), stop=(kc == 3))
                            jb = j * (TPC // P) + tb
                            nc.scalar.copy(v_sb[:, jb * WKK:(jb + 1) * WKK], ps[:])

                for i in range(NCORES):
                    nc.sync.dma_start(qpe[:, i * TPC:(i + 1) * TPC],
                                      qa_out[i * 2 * P: i * 2 * P + P, :])
                    nc.sync.dma_start(qnope[:, i * TPC:(i + 1) * TPC],
                                      qa_out[i * 2 * P + P: (i + 1) * 2 * P, :])
                    nc.sync.dma_start(qnope[:, T + i * TPC: T + (i + 1) * TPC],
                                      qb_out[i * P:(i + 1) * P, :])

                # ============ attention (4 causal units, hl-major) ============
                with tc.tile_pool(name="attc", bufs=1) as attc, \
                     tc.tile_pool(name="att_t", bufs=4) as att_t, \
                     tc.tile_pool(name="ps_s", bufs=4, space="PSUM") as ps_s_pool, \
                     tc.tile_pool(name="ps_o", bufs=2, space="PSUM") as ps_o_pool, \
                     tc.tile_pool(name="ps_d", bufs=2, space="PSUM") as ps_d_pool:
                    mask_sb = attc.tile([P, 4 * 512], BF16)
                    nc.sync.dma_start(mask_sb[:], masks.ap()[:])

                    for u in range(4):  # hl-major: (hl, bb)
                        hl, bb = u // 2, u % 2
                        for qt in range(QT_PER_B):
                            qoff = bb * S + qt * 512
                            ps_o = ps_o_pool.tile([P, 512], F32, tag="pso")
                            ps_d = ps_d_pool.tile([1, 512], F32, tag="psd")
                            nkc = 4 * (qt + 1)
                            for kc in range(nkc):
                                koff = bb * S + kc * P
                                ps_sc = ps_s_pool.tile([P, 512], F32, tag="pss")
                                nc.tensor.matmul(
                                    ps_sc[:], knope[:, hl * T + koff: hl * T + koff + P],
                                    qnope[:, hl * T + qoff: hl * T + qoff + 512],
                                    start=True, stop=False)
                                nc.tensor.matmul(
                                    ps_sc[:], kpe2[hl * DR: hl * DR + DR, koff: koff + P],
                                    qpe[hl * DR: hl * DR + DR, qoff: qoff + 512],
                                    start=False, stop=True)
                                ex = att_t.tile([P, 512], BF16, tag="ex")
                                nc.scalar.activation(ex[:], ps_sc[:], AF.Exp)
                                if kc >= 4 * qt:
                                    mi = kc - 4 * qt
                                    nc.vector.tensor_mul(ex[:], ex[:],
                                                         mask_sb[:, mi * 512:(mi + 1) * 512])
                                jb = bb * KB_PER_B + kc
                                nc.tensor.matmul(
                                    ps_o[:], v_sb[:, jb * WKK + hl * DV: jb * WKK + (hl + 1) * DV],
                                    ex[:], start=(kc == 0), stop=(kc == nkc - 1),
                                    skip_group_check=True)
                                nc.tensor.matmul(
                                    ps_d[:], ones_col[:], ex[:],
                                    start=(kc == 0), stop=(kc == nkc - 1),
                                    skip_group_check=True)
                            # normalize
                            ou = att_t.tile([P, 512], F32, tag="ou")
                            nc.scalar.copy(ou[:], ps_o[:])
                            recip = att_t.tile([1, 512], F32, tag="rcp")
                            nc.vector.reciprocal_approx_fast(recip[:], ps_d[:])
                            bc = ps_s_pool.tile([P, 512], F32, tag="pss")
                            nc.tensor.matmul(bc[:], ones_row[:], recip[:], start=True, stop=True)
                            on = att_t.tile([P, 512], BF16, tag="on")
                            nc.vector.tensor_mul(on[:], ou[:], bc[:])
                            blk = bb * QT_PER_B + qt
                            tgt = oa_in if hl == 0 else ob_in
                            nc.sync.dma_start(tgt[blk * DV:(blk + 1) * DV, :], on[:])
                        if u == 1:  # even heads complete -> overlap with odd attention
                            nc.gpsimd.collective_compute(
                                "AllToAll", mybir.AluOpType.bypass, replica_groups=RG,
                                ins=[oa_in.opt()], outs=[oa_out.opt()])

            p1q_stack.close()
            nc.gpsimd.collective_compute(
                "AllToAll", mybir.AluOpType.bypass, replica_groups=RG,
                ins=[ob_in.opt()], outs=[ob_out.opt()])

            # ============ Phase 3: two passes (pass 1 overlaps the ob AllToAll) ============
            with tc.tile_pool(name="p3w", bufs=1) as p3w, \
                 tc.tile_pool(name="p3t", bufs=3) as p3t, \
                 tc.tile_pool(name="ps3", bufs=4, space="PSUM") as ps3:
                woe_sb = p3w.tile([P, NCORES * HID], BF16)
                for i in range(NCORES):
                    nc.sync.dma_start(woe_sb[:, i * HID:(i + 1) * HID],
                                      woT.ap()[(2 * i) * P:(2 * i + 1) * P, :])
                oe_sb = p3w.tile([P, NCORES * TPC], BF16)
                for i in range(NCORES):
                    nc.sync.dma_start(oe_sb[:, i * TPC:(i + 1) * TPC],
                                      oa_out[i * P:(i + 1) * P, :])
                part_sb = p3w.tile([P, NHID * TPC], F32)
                woo_sb = p3w.tile([P, NCORES * HID], BF16)
                for i in range(NCORES):
                    nc.sync.dma_start(woo_sb[:, i * HID:(i + 1) * HID],
                                      woT.ap()[(2 * i + 1) * P:(2 * i + 2) * P, :])
                oo_sb = p3w.tile([P, NCORES * TPC], BF16)
                for i in range(NCORES):
                    nc.sync.dma_start(oo_sb[:, i * TPC:(i + 1) * TPC],
                                      ob_out[i * P:(i + 1) * P, :])
                for m in range(NHID):
                    ps = ps3.tile([P, TPC], F32, tag="proj")
                    for i in range(NCORES):
                        nc.tensor.matmul(
                            ps[:], woe_sb[:, i * HID + m * P: i * HID + (m + 1) * P],
                            oe_sb[:, i * TPC:(i + 1) * TPC],
                            start=(i == 0), stop=(i == NCORES - 1))
                    nc.scalar.copy(part_sb[:, m * TPC:(m + 1) * TPC], ps[:])
                for m in range(NHID):
                    ps = ps3.tile([P, TPC], F32, tag="proj")
                    for i in range(NCORES):
                        nc.tensor.matmul(
                            ps[:], woo_sb[:, i * HID + m * P: i * HID + (m + 1) * P],
                            oo_sb[:, i * TPC:(i + 1) * TPC],
                            start=(i == 0), stop=(i == NCORES - 1))
                    ot = p3t.tile([P, TPC], F32, tag="ot")
                    nc.vector.tensor_add(ot[:], ps[:], part_sb[:, m * TPC:(m + 1) * TPC])
                    nc.sync.dma_start(outT.ap()[m * P:(m + 1) * P, :], ot[:])
    nc.finalize()
    return nc


def _bf16(x):
    return np.ascontiguousarray(x.astype(ml_dtypes.bfloat16))


def _rope_tables():
    inv_freq = 1.0 / (THETA ** (np.arange(0, DR, 2, dtype=np.float64) / DR))
    t = np.arange(S, dtype=np.float64)
    freqs = np.outer(t, inv_freq)
    emb = np.concatenate((freqs, freqs), axis=-1)
    return np.cos(emb).astype(np.float32), np.sin(emb).astype(np.float32)


def prepare_inputs(hidden_states, w_qa, q_a_ln_w, w_qb, w_kva, kv_a_ln_w, w_kvb, w_o):
    hidden_states = np.asarray(hidden_states, dtype=np.float32)
    w_qa = np.asarray(w_qa, dtype=np.float32)
    q_a_ln_w = np.asarray(q_a_ln_w, dtype=np.float32)
    w_qb = np.asarray(w_qb, dtype=np.float32)
    w_kva = np.asarray(w_kva, dtype=np.float32)
    kv_a_ln_w = np.asarray(kv_a_ln_w, dtype=np.float32)
    w_kvb = np.asarray(w_kvb, dtype=np.float32)
    w_o = np.asarray(w_o, dtype=np.float32)

    flat = hidden_states.reshape(T, HID)
    cos, sin = _rope_tables()          # [S, DR]
    scale = DQ ** -0.5

    pos = np.arange(T) % S
    cos_d = cos[pos].T                 # [DR, T]
    sin_d = sin[pos].T

    kp = np.arange(P)[:, None]
    qf = np.arange(512)[None, :]
    masks = _bf16(np.concatenate(
        [(qf >= kp + P * p).astype(np.float32) for p in range(4)], axis=1))

    w_qb_eff = (w_qb * q_a_ln_w[None, :]) * scale       # [H*DQ, QLR]
    w_kvb_eff = w_kvb * kv_a_ln_w[None, :]              # [H*(DN+DV), KVLR]

    # w_qb rows permuted: block A = per pair j [h0 pe | h1 pe | h0 nope],
    # block B = per pair j [h1 nope]
    rows = []
    for j in range(NCORES):
        h0, h1 = 2 * j, 2 * j + 1
        rows.append(w_qb_eff[h0 * DQ + DN: h0 * DQ + DQ])   # h0 pe (64)
        rows.append(w_qb_eff[h1 * DQ + DN: h1 * DQ + DQ])   # h1 pe (64)
        rows.append(w_qb_eff[h0 * DQ: h0 * DQ + DN])        # h0 nope (128)
    for j in range(NCORES):
        h1 = 2 * j + 1
        rows.append(w_qb_eff[h1 * DQ: h1 * DQ + DN])        # h1 nope (128)
    wqbT_full = _bf16(np.concatenate(rows, axis=0).T)       # [QLR, 3072]

    wqaT = _bf16(w_qa.T)
    wkvaT = _bf16(w_kva.T)
    woT = _bf16(w_o.T)

    in_maps = []
    for c in range(NCORES):
        heads = [HPC * c + h for h in range(HPC)]
        krows = [w_kvb_eff[h * (DN + DV): h * (DN + DV) + DN] for h in heads]
        wkvbkT_c = _bf16(np.concatenate(krows, axis=0).T)
        vrows = [w_kvb_eff[h * (DN + DV) + DN: (h + 1) * (DN + DV)] for h in heads]
        wkvbvT_c = _bf16(np.concatenate(vrows, axis=0).T)

        tok0 = c * TPC
        cosl = cos_d[:, tok0:tok0 + TPC]
        sinl = sin_d[:, tok0:tok0 + TPC]
        in_maps.append({
            "hidT": _bf16(flat[tok0:tok0 + TPC].T),
            "wqaT": wqaT, "wkvaT": wkvaT,
            "wqbT": wqbT_full, "wkvbkT": wkvbkT_c, "wkvbvT": wkvbvT_c,
            "woT": woT,
            "cosd": np.ascontiguousarray(np.concatenate([cosl, cosl], axis=0)),
            "sind": np.ascontiguousarray(np.concatenate([sinl, sinl], axis=0)),
            "masks": masks,
        })
    return in_maps


def kernel(hidden_states, w_qa, q_a_ln_w, w_qb, w_kva, kv_a_ln_w, w_kvb, w_o,
           _trace=False):
    global _NC_CACHE
    if _NC_CACHE is None:
        _NC_CACHE = build_nc()
    nc = _NC_CACHE
    in_maps = prepare_inputs(hidden_states, w_qa, q_a_ln_w, w_qb, w_kva,
                             kv_a_ln_w, w_kvb, w_o)
    res = run_bass_kernel_spmd(nc, in_maps, core_ids=list(range(NCORES)),
                               trace=_trace)
    out = np.empty((T, HID), dtype=np.float32)
    for c in range(NCORES):
        out[c * TPC:(c + 1) * TPC] = res.results[c]["outT"].T
    if _trace:
        kernel._last_result = res
    return out.reshape(B, S, HID)



# revision 19
# speedup vs baseline: 1.1535x; 1.1535x over previous
"""MLA (DeepSeek-style) attention block on 8 Trainium2 NeuronCores.

Sharding:
  phase 1 (token-parallel, 8 x 512 tokens): LoRA-A down-projections + rmsnorm
    + k_pe rope; small AllGather of the kv latents (576 dims); q up-projection
    for ALL 16 heads on the token side + rope, shipped to head shards via two
    shard-aligned AllToAlls (pe+even-nope first, odd-nope second).
  phase 2 (head-parallel, 2 heads x 2 batches per core): k/v up-projection
    from gathered kv latents; causal flash attention (k-major scores, exp on
    ScalarE, ones-matmul denominator, reciprocal + K=1 broadcast matmul for
    the divide).
  output: two AllToAlls (even heads overlap the odd-head attention; odd heads
    overlap the even half of the token-parallel output projection).

Scheduling notes (v2):
  - DMA issue order keeps the first ckv matmul start at ~1.5us (hid/wkva
    interleaved first; wqb prefetch rides the second HWDGE queue).
  - ckv and cq run kc-outer (accumulate all m-blocks per hid chunk) so
    compute starts before the weight streams finish.
  - kv up-projection runs AFTER the q up-projection so the q AllToAlls hide
    behind it.
  - attention is software-pipelined with a 2-chunk lookahead (PV/denominator
    of chunk i issue after the scores of chunk i+2) and per-q-tile
    normalization is deferred by one chunk; diagonal blocks only compute the
    causally visible column range.
  - o_proj weights prefetch during attention; output AllToAll receives issue
    right after each collective.

bf16 matmuls, fp32 PSUM accumulation + softmax statistics, fp32 output.
"""
import sys
from collections import deque
from contextlib import ExitStack

sys.path.insert(0, "/opt/trn_rl_repo")

import numpy as np
import ml_dtypes

import concourse.bacc as bacc
import concourse.mybir as mybir
import concourse.tile as tile
from concourse.bass_utils import run_bass_kernel_spmd

# ---- problem sizes (hardcoded per spec) ----
HID = 2048; H = 16; QLR = 1536; KVLR = 512
DN = 128; DR = 64; DV = 128; DQ = DN + DR
B = 2; S = 2048
THETA = 10000.0; EPS = 1e-6

NCORES = 8
T = B * S              # 4096 flattened tokens
TPC = T // NCORES      # 512 tokens per core
HPC = H // NCORES      # 2 heads per core
P = 128
NHID = HID // P        # 16
NQLR = QLR // P        # 12
CKW = KVLR + DR        # 576
QT_PER_B = S // 512    # 4 q-tiles of 512 per (b,h) unit
KB_PER_B = S // P      # 16 k-chunks of 128 per batch
WKK = HPC * DN         # 256

BF16 = mybir.dt.bfloat16
F32 = mybir.dt.float32
AF = mybir.ActivationFunctionType

_NC_CACHE = None


def _rope_dual(nc, pool, out_bf16, ps, cos_sb, sin_sb, tag):
    """RoPE on a [128, W] psum holding two 64-row head groups; writes bf16."""
    W = 512
    HDR = DR // 2
    rot = pool.tile([P, W], F32, tag=f"{tag}rot", name=f"{tag}rot")
    for g in range(2):
        o = g * DR
        nc.scalar.mul(rot[o:o + HDR, :], ps[o + HDR:o + DR, :], -1.0)
        nc.scalar.copy(rot[o + HDR:o + DR, :], ps[o:o + HDR, :])
    t1 = pool.tile([P, W], F32, tag=f"{tag}t1", name=f"{tag}t1")
    nc.vector.tensor_mul(t1[:], ps[:], cos_sb[:])
    nc.vector.tensor_mul(rot[:], rot[:], sin_sb[:])
    nc.vector.tensor_add(out_bf16[:], t1[:], rot[:])


def build_nc():
    nc = bacc.Bacc(None, target_bir_lowering=False, debug=False, num_devices=NCORES)

    # ---- per-core external inputs ----
    hidT = nc.dram_tensor("hidT", [HID, TPC], BF16, kind="ExternalInput")
    wqaT = nc.dram_tensor("wqaT", [HID, QLR], BF16, kind="ExternalInput")
    wkvaT = nc.dram_tensor("wkvaT", [HID, CKW], BF16, kind="ExternalInput")
    wqbT = nc.dram_tensor("wqbT", [QLR, H * DQ], BF16, kind="ExternalInput")
    wkvbkT = nc.dram_tensor("wkvbkT", [KVLR, HPC * DN], BF16, kind="ExternalInput")
    wkvbvT = nc.dram_tensor("wkvbvT", [KVLR, HPC * DV], BF16, kind="ExternalInput")
    woT = nc.dram_tensor("woT", [H * DV, HID], BF16, kind="ExternalInput")
    cosd = nc.dram_tensor("cosd", [P, TPC], F32, kind="ExternalInput")
    sind = nc.dram_tensor("sind", [P, TPC], F32, kind="ExternalInput")
    masks = nc.dram_tensor("masks", [P, 4 * 512], BF16, kind="ExternalInput")
    outT = nc.dram_tensor("outT", [HID, TPC], F32, kind="ExternalOutput")

    RG = [list(range(NCORES))]

    with tile.TileContext(nc) as tc:
        with tc.tile_pool(name="dram", bufs=1, space="DRAM") as dram, \
             tc.tile_pool(name="const", bufs=1) as const:
            latkv_in = dram.tile([CKW, TPC], BF16)
            latkv_all = dram.tile([NCORES * CKW, TPC], BF16, addr_space="Shared")
            qa_in = dram.tile([NCORES * 2 * P, TPC], BF16)   # [pe2|h0n] per pair
            qa_out = dram.tile([NCORES * 2 * P, TPC], BF16)
            qb_in = dram.tile([NCORES * P, TPC], BF16)       # h1n per pair
            qb_out = dram.tile([NCORES * P, TPC], BF16)
            oa_in = dram.tile([NCORES * DV, TPC], BF16)      # even heads out
            oa_out = dram.tile([NCORES * DV, TPC], BF16)
            ob_in = dram.tile([NCORES * DV, TPC], BF16)      # odd heads out
            ob_out = dram.tile([NCORES * DV, TPC], BF16)

            ones_col = const.tile([P, 1], BF16)
            nc.vector.memset(ones_col[:], 1.0)
            ones_row = const.tile([1, P], BF16)
            nc.vector.memset(ones_row[:], 1.0)
            eps_t = const.tile([1, 1], F32)
            nc.vector.memset(eps_t[:], EPS)
            cos_sb = const.tile([P, TPC], F32)
            sin_sb = const.tile([P, TPC], F32)
            mask_sb = const.tile([P, 512], BF16)

            # ============ phase-1 pools (LIFO: p1q outer, p1w innermost) ============
            p1q_stack = ExitStack()
            p1q = p1q_stack.enter_context(tc.tile_pool(name="p1q", bufs=1))
            p1t_stack = ExitStack()
            p1t = p1t_stack.enter_context(tc.tile_pool(name="p1t", bufs=1))
            ps1_stack = ExitStack()
            ps1 = ps1_stack.enter_context(tc.tile_pool(name="ps1", bufs=6, space="PSUM"))
            ps1s = ps1_stack.enter_context(tc.tile_pool(name="ps1s", bufs=1, space="PSUM"))
            ps1b = ps1_stack.enter_context(tc.tile_pool(name="ps1b", bufs=1, space="PSUM"))
            p1w_stack = ExitStack()
            p1w = p1w_stack.enter_context(tc.tile_pool(name="p1w", bufs=1))

            hid_ch = [p1w.tile([P, TPC], BF16, tag=f"hid{kc}", name=f"hid{kc}")
                      for kc in range(NHID)]
            wkva_ch = [p1w.tile([P, CKW], BF16, tag=f"wkva{kc}", name=f"wkva{kc}")
                       for kc in range(NHID)]
            wqa_ch = [p1w.tile([P, QLR], BF16, tag=f"wqa{kc}", name=f"wqa{kc}")
                      for kc in range(NHID)]
            WQB = H * DQ  # 3072
            wqb_ch = [p1q.tile([P, WQB], BF16, tag=f"wqb{kc}", name=f"wqb{kc}")
                      for kc in range(NQLR)]
            cqn_sb = p1q.tile([P, NQLR * TPC], BF16)

            # ---- DMA issue order (sync queue): compute-critical first ----
            nc.sync.dma_start(hid_ch[0][:], hidT.ap()[0:P, :])
            nc.sync.dma_start(wkva_ch[0][:], wkvaT.ap()[0:P, :])
            nc.sync.dma_start(cos_sb[:], cosd.ap()[:])
            nc.sync.dma_start(sin_sb[:], sind.ap()[:])
            for kc in range(1, NHID):
                nc.sync.dma_start(hid_ch[kc][:], hidT.ap()[kc * P:(kc + 1) * P, :])
                nc.sync.dma_start(wkva_ch[kc][:], wkvaT.ap()[kc * P:(kc + 1) * P, :])
            for kc in range(NHID):
                nc.sync.dma_start(wqa_ch[kc][:], wqaT.ap()[kc * P:(kc + 1) * P, :])
            # second HWDGE queue (scalar engine): bulk prefetch
            nc.scalar.dma_start(mask_sb[:], masks.ap()[:, 0:512])
            for kc in range(NQLR):
                nc.scalar.dma_start(wqb_ch[kc][:], wqbT.ap()[kc * P:(kc + 1) * P, :])

            # ============ ckv joint projection (kc-outer) ============
            ps_ckv = [ps1.tile([P, TPC], F32, tag="proj", name=f"psckv{m}")
                      for m in range(4)]
            ps_pe = ps1.tile([DR, TPC], F32, tag="proj")
            for kc in range(NHID):
                for m in range(4):
                    nc.tensor.matmul(ps_ckv[m][:], wkva_ch[kc][:, m * P:(m + 1) * P],
                                     hid_ch[kc][:],
                                     start=(kc == 0), stop=(kc == NHID - 1),
                                     skip_group_check=True)
                nc.tensor.matmul(ps_pe[:], wkva_ch[kc][:, KVLR:CKW], hid_ch[kc][:],
                                 start=(kc == 0), stop=(kc == NHID - 1),
                                 skip_group_check=True)

            # k_pe rope (shared across heads) -> latkv_in[512:576]
            HDR = DR // 2
            rot = p1t.tile([DR, TPC], F32, tag="rot")
            nc.scalar.mul(rot[0:HDR, :], ps_pe[HDR:DR, :], -1.0)
            nc.scalar.copy(rot[HDR:DR, :], ps_pe[0:HDR, :])
            t1 = p1t.tile([DR, TPC], F32, tag="t1")
            nc.vector.tensor_mul(t1[:], ps_pe[:], cos_sb[0:DR, :])
            nc.vector.tensor_mul(rot[:], rot[:], sin_sb[0:DR, :])
            pe_out = p1t.tile([DR, TPC], BF16, tag="peo")
            nc.vector.tensor_add(pe_out[:], t1[:], rot[:])
            nc.sync.dma_start(latkv_in[KVLR:CKW, :], pe_out[:])

            # ckv copies + squares (scalar/vector; run during cq passA below)
            ckv_bf = p1t.tile([P, 4 * TPC], BF16)
            sq_kv = [p1t.tile([P, TPC], BF16, tag="sq", name=f"sqkv{m}", bufs=6)
                     for m in range(4)]
            for m in range(4):
                nc.scalar.copy(ckv_bf[:, m * TPC:(m + 1) * TPC], ps_ckv[m][:])
                nc.vector.tensor_mul(sq_kv[m][:], ckv_bf[:, m * TPC:(m + 1) * TPC],
                                     ckv_bf[:, m * TPC:(m + 1) * TPC])

            # ============ cq pass A (m=0..5, kc-outer) ============
            def cq_block(mlist, kc):
                for mi, m in enumerate(mlist):
                    nc.tensor.matmul(ps_cq[mi][:], wqa_ch[kc][:, m * P:(m + 1) * P],
                                     hid_ch[kc][:],
                                     start=(kc == 0), stop=(kc == NHID - 1),
                                     skip_group_check=True)

            mlistA = list(range(6))
            ps_cq = [ps1.tile([P, TPC], F32, tag="proj", name=f"pscqa{m}")
                     for m in range(6)]
            cq_block(mlistA, 0)
            cq_block(mlistA, 1)
            # ssq_kv accumulation (tensor; deps on sq_kv fall in this window)
            ssq_kv = ps1s.tile([1, TPC], F32, tag="ssq")
            for m in range(4):
                nc.tensor.matmul(ssq_kv[:], ones_col[:], sq_kv[m][:],
                                 start=(m == 0), stop=(m == 3),
                                 skip_group_check=True)
            cq_block(mlistA, 2)
            # kv rmsnorm chain + broadcast
            kv_norm = p1t.tile([1, TPC], F32, tag="nrm")
            nc.scalar.activation(kv_norm[:], ssq_kv[:], AF.Sqrt, bias=eps_t[:],
                                 scale=1.0 / KVLR)
            rn_kv = p1t.tile([1, TPC], F32, tag="rn")
            nc.vector.reciprocal(rn_kv[:], kv_norm[:])
            rn_kv_bf = p1t.tile([1, TPC], BF16, tag="rnb")
            nc.scalar.copy(rn_kv_bf[:], rn_kv[:])
            bkv = ps1b.tile([P, TPC], F32, tag="bc")
            nc.tensor.matmul(bkv[:], ones_row[:], rn_kv_bf[:], start=True, stop=True,
                             skip_group_check=True)
            for m in range(4):
                lat_sb = p1t.tile([P, TPC], BF16, tag="lat", bufs=2)
                nc.vector.tensor_mul(lat_sb[:], ckv_bf[:, m * TPC:(m + 1) * TPC], bkv[:])
                nc.sync.dma_start(latkv_in[m * P:(m + 1) * P, :], lat_sb[:])
            nc.gpsimd.collective_compute(
                "AllGather", mybir.AluOpType.bypass, replica_groups=RG,
                ins=[latkv_in.opt()], outs=[latkv_all.opt()])
            for kc in range(3, NHID):
                cq_block(mlistA, kc)
            # copies + squares for pass A blocks (run during pass B)
            sq_q = {}
            for mi, m in enumerate(mlistA):
                sq_q[m] = p1t.tile([P, TPC], BF16, tag="sq", name=f"sqq{m}", bufs=6)
                nc.scalar.copy(cqn_sb[:, m * TPC:(m + 1) * TPC], ps_cq[mi][:])
                nc.vector.tensor_mul(sq_q[m][:], cqn_sb[:, m * TPC:(m + 1) * TPC],
                                     cqn_sb[:, m * TPC:(m + 1) * TPC])

            # ============ cq pass B (m=6..11, kc-outer) ============
            mlistB = list(range(6, 12))
            ps_cq = [ps1.tile([P, TPC], F32, tag="proj", name=f"pscqb{m}")
                     for m in range(6)]
            cq_block(mlistB, 0)
            cq_block(mlistB, 1)
            ssq_q = ps1s.tile([1, TPC], F32, tag="ssq")
            for m in mlistA:
                nc.tensor.matmul(ssq_q[:], ones_col[:], sq_q[m][:],
                                 start=(m == 0), stop=False,
                                 skip_group_check=True)
            for kc in range(2, NHID):
                cq_block(mlistB, kc)
            for mi, m in enumerate(mlistB):
                sq_q[m] = p1t.tile([P, TPC], BF16, tag="sq", name=f"sqqb{m}", bufs=6)
                nc.scalar.copy(cqn_sb[:, m * TPC:(m + 1) * TPC], ps_cq[mi][:])
                nc.vector.tensor_mul(sq_q[m][:], cqn_sb[:, m * TPC:(m + 1) * TPC],
                                     cqn_sb[:, m * TPC:(m + 1) * TPC])
                nc.tensor.matmul(ssq_q[:], ones_col[:], sq_q[m][:],
                                 start=False, stop=(m == 11),
                                 skip_group_check=True)
            sq_norm = p1t.tile([1, TPC], F32, tag="nrm")
            nc.scalar.activation(sq_norm[:], ssq_q[:], AF.Sqrt, bias=eps_t[:],
                                 scale=1.0 / QLR)
            rn_q = p1t.tile([1, TPC], F32, tag="rn")
            nc.vector.reciprocal(rn_q[:], sq_norm[:])
            rn_q_bf = p1t.tile([1, TPC], BF16, tag="rnb")
            nc.scalar.copy(rn_q_bf[:], rn_q[:])
            bq = ps1b.tile([P, TPC], F32, tag="bc")
            nc.tensor.matmul(bq[:], ones_row[:], rn_q_bf[:], start=True, stop=True,
                             skip_group_check=True)
            for m in range(NQLR):
                nc.vector.tensor_mul(cqn_sb[:, m * TPC:(m + 1) * TPC],
                                     cqn_sb[:, m * TPC:(m + 1) * TPC], bq[:])
            p1w_stack.close()  # free hid/wkva/wqa SBUF

            # ============ q up-projection for ALL heads ============
            with tc.tile_pool(name="p1qt", bufs=3) as p1qt:
                for mb in range(16):
                    ps = ps1.tile([P, TPC], F32, tag="proj")
                    for kc in range(NQLR):
                        nc.tensor.matmul(ps[:], wqb_ch[kc][:, mb * P:(mb + 1) * P],
                                         cqn_sb[:, kc * TPC:(kc + 1) * TPC],
                                         start=(kc == 0), stop=(kc == NQLR - 1),
                                         skip_group_check=True)
                    qo = p1qt.tile([P, TPC], BF16, tag="qo")
                    if mb % 2 == 0:  # pe2 block -> rope
                        _rope_dual(nc, p1qt, qo, ps, cos_sb, sin_sb, "q")
                    else:
                        nc.scalar.copy(qo[:], ps[:])
                    nc.sync.dma_start(qa_in[mb * P:(mb + 1) * P, :], qo[:])
                nc.gpsimd.collective_compute(
                    "AllToAll", mybir.AluOpType.bypass, replica_groups=RG,
                    ins=[qa_in.opt()], outs=[qa_out.opt()])
                for mb in range(8):
                    ps = ps1.tile([P, TPC], F32, tag="proj")
                    for kc in range(NQLR):
                        nc.tensor.matmul(ps[:], wqb_ch[kc][:, (16 + mb) * P:(17 + mb) * P],
                                         cqn_sb[:, kc * TPC:(kc + 1) * TPC],
                                         start=(kc == 0), stop=(kc == NQLR - 1),
                                         skip_group_check=True)
                    qo = p1qt.tile([P, TPC], BF16, tag="qo")
                    nc.scalar.copy(qo[:], ps[:])
                    nc.sync.dma_start(qb_in[mb * P:(mb + 1) * P, :], qo[:])
                nc.gpsimd.collective_compute(
                    "AllToAll", mybir.AluOpType.bypass, replica_groups=RG,
                    ins=[qb_in.opt()], outs=[qb_out.opt()])
            ps1_stack.close()
            p1t_stack.close()
            p1q_stack.close()

            # ===== phase 2: k/v up-projection (overlaps the q AllToAlls) =====
            att_a_stack = ExitStack()
            att_a = att_a_stack.enter_context(tc.tile_pool(name="att_a", bufs=1))
            knope = att_a.tile([P, 2 * T], BF16)
            kpe2 = att_a.tile([P, T], BF16)    # k_pe duplicated rows
            v_sb = att_a.tile([P, (T // P) * WKK], BF16)
            qnope = att_a.tile([P, 2 * T], BF16)
            qpe = att_a.tile([P, T], BF16)     # rows 0-63 h0, 64-127 h1

            with tc.tile_pool(name="p2a", bufs=4) as p2a, \
                 tc.tile_pool(name="ps2", bufs=4, space="PSUM") as ps2:
                wkk_sb = p2a.tile([P, 4 * WKK], BF16, tag="wkk", bufs=1)
                wkv_sb = p2a.tile([P, 4 * WKK], BF16, tag="wkv", bufs=1)
                for kc in range(4):
                    nc.sync.dma_start(wkk_sb[:, kc * WKK:(kc + 1) * WKK],
                                      wkvbkT.ap()[kc * P:(kc + 1) * P, :])
                    nc.sync.dma_start(wkv_sb[:, kc * WKK:(kc + 1) * WKK],
                                      wkvbvT.ap()[kc * P:(kc + 1) * P, :])
                for j in range(NCORES):
                    basek = j * CKW
                    ckv_j = p2a.tile([P, 4 * TPC], BF16, tag="ckvj")
                    for r in range(4):
                        nc.sync.dma_start(ckv_j[:, r * TPC:(r + 1) * TPC],
                                          latkv_all[basek + r * P: basek + (r + 1) * P, :])
                    nc.sync.dma_start(kpe2[0:DR, j * TPC:(j + 1) * TPC],
                                      latkv_all[basek + KVLR: basek + CKW, :])
                    nc.sync.dma_start(kpe2[DR:P, j * TPC:(j + 1) * TPC],
                                      latkv_all[basek + KVLR: basek + CKW, :])
                    for m in range(HPC):
                        ps = ps2.tile([P, TPC], F32, tag="proj")
                        for kc in range(4):
                            nc.tensor.matmul(
                                ps[:], wkk_sb[:, kc * WKK + m * P: kc * WKK + (m + 1) * P],
                                ckv_j[:, kc * TPC:(kc + 1) * TPC],
                                start=(kc == 0), stop=(kc == 3))
                        nc.scalar.copy(knope[:, m * T + j * TPC: m * T + (j + 1) * TPC], ps[:])
                    for tb in range(TPC // P):
                        ps = ps2.tile([P, WKK], F32, tag="projv")
                        for kc in range(4):
                            nc.tensor.matmul(
                                ps[:], ckv_j[:, kc * TPC + tb * P: kc * TPC + (tb + 1) * P],
                                wkv_sb[:, kc * WKK:(kc + 1) * WKK],
                                start=(kc == 0), stop=(kc == 3))
                        jb = j * (TPC // P) + tb
                        nc.scalar.copy(v_sb[:, jb * WKK:(jb + 1) * WKK], ps[:])
            # q receives: qa-dependent first, then qb-dependent
            for i in range(NCORES):
                nc.sync.dma_start(qpe[:, i * TPC:(i + 1) * TPC],
                                  qa_out[i * 2 * P: i * 2 * P + P, :])
                nc.sync.dma_start(qnope[:, i * TPC:(i + 1) * TPC],
                                  qa_out[i * 2 * P + P: (i + 1) * 2 * P, :])
            for i in range(NCORES):
                nc.sync.dma_start(qnope[:, T + i * TPC: T + (i + 1) * TPC],
                                  qb_out[i * P:(i + 1) * P, :])

            # o_proj weight prefetch (transfers run during attention)
            p3w_stack = ExitStack()
            p3w = p3w_stack.enter_context(tc.tile_pool(name="p3w", bufs=1))
            woe_sb = p3w.tile([P, NCORES * HID], BF16)
            woo_sb = p3w.tile([P, NCORES * HID], BF16)
            oe_sb = p3w.tile([P, NCORES * TPC], BF16)
            oo_sb = p3w.tile([P, NCORES * TPC], BF16)
            part_sb = p3w.tile([P, NHID * TPC], F32)
            for i in range(NCORES):
                nc.sync.dma_start(woe_sb[:, i * HID:(i + 1) * HID],
                                  woT.ap()[(2 * i) * P:(2 * i + 1) * P, :])
            for i in range(NCORES):
                nc.sync.dma_start(woo_sb[:, i * HID:(i + 1) * HID],
                                  woT.ap()[(2 * i + 1) * P:(2 * i + 2) * P, :])

            # ============ attention: software-pipelined, causal-trimmed ============
            with tc.tile_pool(name="att_t", bufs=1) as att_t, \
                 tc.tile_pool(name="ps_s", bufs=3, space="PSUM") as ps_s_pool, \
                 tc.tile_pool(name="ps_o", bufs=2, space="PSUM") as ps_o_pool, \
                 tc.tile_pool(name="ps_d", bufs=2, space="PSUM") as ps_d_pool, \
                 tc.tile_pool(name="ps_b", bufs=1, space="PSUM") as ps_b_pool:

                tasks = []
                for u in range(4):          # hl-major: (hl, bb)
                    hl, bb = u // 2, u % 2
                    for qt in range(QT_PER_B):
                        nkc = 4 * (qt + 1)
                        for kc in range(nkc):
                            tasks.append((u, hl, bb, qt, kc, kc == nkc - 1))

                state = {}  # per (u,qt): dict(ps_o, ps_d, ex tiles keyed by kc)
                ex_of = {}  # task idx -> (ex tile, m0)
                norm_pending = None

                def issue_scores(i, t):
                    u, hl, bb, qt, kc, _ = t
                    qoff = bb * S + qt * 512
                    koff = bb * S + kc * P
                    mi = kc - 4 * qt
                    m0 = mi * P if mi >= 0 else 0
                    ps_sc = ps_s_pool.tile([P, 512], F32, tag="pss")
                    nc.tensor.matmul(
                        ps_sc[:, m0:512], knope[:, hl * T + koff: hl * T + koff + P],
                        qnope[:, hl * T + qoff + m0: hl * T + qoff + 512],
                        start=True, stop=False, skip_group_check=True)
                    nc.tensor.matmul(
                        ps_sc[:, m0:512], kpe2[hl * DR: hl * DR + DR, koff: koff + P],
                        qpe[hl * DR: hl * DR + DR, qoff + m0: qoff + 512],
                        start=False, stop=True, skip_group_check=True)
                    ex = att_t.tile([P, 512], BF16, tag="ex", bufs=4)
                    nc.scalar.activation(ex[:, m0:512], ps_sc[:, m0:512], AF.Exp)
                    if mi >= 0:
                        nc.vector.tensor_mul(ex[:, m0:m0 + P], ex[:, m0:m0 + P],
                                             mask_sb[:, 0:P])
                    ex_of[i] = (ex, m0)

                def issue_pv(i, t):
                    """PV + denominator for task i; returns norm info if qt closed."""
                    u, hl, bb, qt, kc, is_last = t
                    ex, m0 = ex_of.pop(i)
                    key = (u, qt)
                    if key not in state:
                        state[key] = {
                            "ps_o": ps_o_pool.tile([P, 512], F32, tag="pso",
                                                   name="pso"),
                            "ps_d": ps_d_pool.tile([1, 512], F32, tag="psd",
                                                   name="psd"),
                        }
                    st = state[key]
                    jb = bb * KB_PER_B + kc
                    nc.tensor.matmul(
                        st["ps_o"][:, m0:512],
                        v_sb[:, jb * WKK + hl * DV: jb * WKK + (hl + 1) * DV],
                        ex[:, m0:512], start=(kc == 0), stop=is_last,
                        skip_group_check=True)
                    nc.tensor.matmul(
                        st["ps_d"][:, m0:512], ones_col[:], ex[:, m0:512],
                        start=(kc == 0), stop=is_last, skip_group_check=True)
                    if is_last:
                        recip = att_t.tile([1, 512], F32, tag="rcp", bufs=2)
                        nc.vector.reciprocal_approx_fast(recip[:], st["ps_d"][:])
                        recip_bf = att_t.tile([1, 512], BF16, tag="rcpb", bufs=2)
                        nc.scalar.copy(recip_bf[:], recip[:])
                        return (u, hl, bb, qt, st, recip_bf)
                    return None

                def issue_norm(info):
                    u, hl, bb, qt, st, recip = info
                    bc = ps_b_pool.tile([P, 512], F32, tag="bc")
                    nc.tensor.matmul(bc[:], ones_row[:], recip[:], start=True,
                                     stop=True, skip_group_check=True)
                    ou = att_t.tile([P, 512], F32, tag="ou", bufs=2)
                    nc.scalar.copy(ou[:], st["ps_o"][:])
                    on = att_t.tile([P, 512], BF16, tag="on", bufs=2)
                    nc.vector.tensor_mul(on[:], ou[:], bc[:])
                    blk = bb * QT_PER_B + qt
                    tgt = oa_in if hl == 0 else ob_in
                    nc.sync.dma_start(tgt[blk * DV:(blk + 1) * DV, :], on[:])
                    del state[(u, qt)]
                    if u == 1 and qt == QT_PER_B - 1:
                        # even heads complete -> ship while odd attention runs
                        nc.gpsimd.collective_compute(
                            "AllToAll", mybir.AluOpType.bypass, replica_groups=RG,
                            ins=[oa_in.opt()], outs=[oa_out.opt()])
                        for i2 in range(NCORES):
                            nc.sync.dma_start(oe_sb[:, i2 * TPC:(i2 + 1) * TPC],
                                              oa_out[i2 * P:(i2 + 1) * P, :])

                LOOKAHEAD = 2
                pend = deque()
                for i, t in enumerate(tasks):
                    issue_scores(i, t)
                    if norm_pending is not None:
                        issue_norm(norm_pending)
                        norm_pending = None
                    if len(pend) == LOOKAHEAD:
                        pi, pt = pend.popleft()
                        norm_pending = issue_pv(pi, pt)
                    pend.append((i, t))
                while pend:
                    if norm_pending is not None:
                        issue_norm(norm_pending)
                        norm_pending = None
                    pi, pt = pend.popleft()
                    norm_pending = issue_pv(pi, pt)
                if norm_pending is not None:
                    issue_norm(norm_pending)

            nc.gpsimd.collective_compute(
                "AllToAll", mybir.AluOpType.bypass, replica_groups=RG,
                ins=[ob_in.opt()], outs=[ob_out.opt()])
            for i in range(NCORES):
                nc.sync.dma_start(oo_sb[:, i * TPC:(i + 1) * TPC],
                                  ob_out[i * P:(i + 1) * P, :])

            # ============ phase 3: o_proj (pass 1 overlaps the ob AllToAll) ============
            with tc.tile_pool(name="p3t", bufs=3) as p3t, \
                 tc.tile_pool(name="ps3", bufs=4, space="PSUM") as ps3:
                for m in range(NHID):
                    ps = ps3.tile([P, TPC], F32, tag="proj")
                    for i in range(NCORES):
                        nc.tensor.matmul(
                            ps[:], woe_sb[:, i * HID + m * P: i * HID + (m + 1) * P],
                            oe_sb[:, i * TPC:(i + 1) * TPC],
                            start=(i == 0), stop=(i == NCORES - 1))
                    nc.scalar.copy(part_sb[:, m * TPC:(m + 1) * TPC], ps[:])
                for m in range(NHID):
                    ps = ps3.tile([P, TPC], F32, tag="proj")
                    for i in range(NCORES):
                        nc.tensor.matmul(
                            ps[:], woo_sb[:, i * HID + m * P: i * HID + (m + 1) * P],
                            oo_sb[:, i * TPC:(i + 1) * TPC],
                            start=(i == 0), stop=(i == NCORES - 1))
                    ot = p3t.tile([P, TPC], F32, tag="ot")
                    nc.vector.tensor_add(ot[:], ps[:], part_sb[:, m * TPC:(m + 1) * TPC])
                    nc.sync.dma_start(outT.ap()[m * P:(m + 1) * P, :], ot[:])
            p3w_stack.close()
            att_a_stack.close()
    nc.finalize()
    return nc


def _bf16(x):
    return np.ascontiguousarray(x.astype(ml_dtypes.bfloat16))


def _rope_tables():
    inv_freq = 1.0 / (THETA ** (np.arange(0, DR, 2, dtype=np.float64) / DR))
    t = np.arange(S, dtype=np.float64)
    freqs = np.outer(t, inv_freq)
    emb = np.concatenate((freqs, freqs), axis=-1)
    return np.cos(emb).astype(np.float32), np.sin(emb).astype(np.float32)


def prepare_inputs(hidden_states, w_qa, q_a_ln_w, w_qb, w_kva, kv_a_ln_w, w_kvb, w_o):
    hidden_states = np.asarray(hidden_states, dtype=np.float32)
    w_qa = np.asarray(w_qa, dtype=np.float32)
    q_a_ln_w = np.asarray(q_a_ln_w, dtype=np.float32)
    w_qb = np.asarray(w_qb, dtype=np.float32)
    w_kva = np.asarray(w_kva, dtype=np.float32)
    kv_a_ln_w = np.asarray(kv_a_ln_w, dtype=np.float32)
    w_kvb = np.asarray(w_kvb, dtype=np.float32)
    w_o = np.asarray(w_o, dtype=np.float32)

    flat = hidden_states.reshape(T, HID)
    cos, sin = _rope_tables()          # [S, DR]
    scale = DQ ** -0.5

    pos = np.arange(T) % S
    cos_d = cos[pos].T                 # [DR, T]
    sin_d = sin[pos].T

    kp = np.arange(P)[:, None]
    qf = np.arange(512)[None, :]
    masks = _bf16(np.concatenate(
        [(qf >= kp + P * p).astype(np.float32) for p in range(4)], axis=1))

    w_qb_eff = (w_qb * q_a_ln_w[None, :]) * scale       # [H*DQ, QLR]
    w_kvb_eff = w_kvb * kv_a_ln_w[None, :]              # [H*(DN+DV), KVLR]

    # w_qb rows permuted: block A = per pair j [h0 pe | h1 pe | h0 nope],
    # block B = per pair j [h1 nope]
    rows = []
    for j in range(NCORES):
        h0, h1 = 2 * j, 2 * j + 1
        rows.append(w_qb_eff[h0 * DQ + DN: h0 * DQ + DQ])   # h0 pe (64)
        rows.append(w_qb_eff[h1 * DQ + DN: h1 * DQ + DQ])   # h1 pe (64)
        rows.append(w_qb_eff[h0 * DQ: h0 * DQ + DN])        # h0 nope (128)
    for j in range(NCORES):
        h1 = 2 * j + 1
        rows.append(w_qb_eff[h1 * DQ: h1 * DQ + DN])        # h1 nope (128)
    wqbT_full = _bf16(np.concatenate(rows, axis=0).T)       # [QLR, 3072]

    wqaT = _bf16(w_qa.T)
    wkvaT = _bf16(w_kva.T)
    woT = _bf16(w_o.T)

    in_maps = []
    for c in range(NCORES):
        heads = [HPC * c + h for h in range(HPC)]
        krows = [w_kvb_eff[h * (DN + DV): h * (DN + DV) + DN] for h in heads]
        wkvbkT_c = _bf16(np.concatenate(krows, axis=0).T)
        vrows = [w_kvb_eff[h * (DN + DV) + DN: (h + 1) * (DN + DV)] for h in heads]
        wkvbvT_c = _bf16(np.concatenate(vrows, axis=0).T)

        tok0 = c * TPC
        cosl = cos_d[:, tok0:tok0 + TPC]
        sinl = sin_d[:, tok0:tok0 + TPC]
        in_maps.append({
            "hidT": _bf16(flat[tok0:tok0 + TPC].T),
            "wqaT": wqaT, "wkvaT": wkvaT,
            "wqbT": wqbT_full, "wkvbkT": wkvbkT_c, "wkvbvT": wkvbvT_c,
            "woT": woT,
            "cosd": np.ascontiguousarray(np.concatenate([cosl, cosl], axis=0)),
            "sind": np.ascontiguousarray(np.concatenate([sinl, sinl], axis=0)),
            "masks": masks,
        })
    return in_maps


def kernel(hidden_states, w_qa, q_a_ln_w, w_qb, w_kva, kv_a_ln_w, w_kvb, w_o,
           _trace=False):
    global _NC_CACHE
    if _NC_CACHE is None:
        _NC_CACHE = build_nc()
    nc = _NC_CACHE
    in_maps = prepare_inputs(hidden_states, w_qa, q_a_ln_w, w_qb, w_kva,
                             kv_a_ln_w, w_kvb, w_o)
    res = run_bass_kernel_spmd(nc, in_maps, core_ids=list(range(NCORES)),
                               trace=_trace)
    out = np.empty((T, HID), dtype=np.float32)
    for c in range(NCORES):
        out[c * TPC:(c + 1) * TPC] = res.results[c]["outT"].T
    if _trace:
        kernel._last_result = res
    return out.reshape(B, S, HID)


# revision 21
# speedup vs baseline: 1.2219x; 1.0593x over previous
"""MLA (DeepSeek-style) attention block on 8 Trainium2 NeuronCores.

Sharding:
  phase 1 (token-parallel, 8 x 512 tokens): LoRA-A down-projections + rmsnorm
    + k_pe rope; small AllGather of the kv latents (576 dims); q up-projection
    for ALL 16 heads on the token side + rope, shipped to head shards via two
    shard-aligned AllToAlls (pe+even-nope first, odd-nope second).
  phase 2 (head-parallel, 2 heads x 2 batches per core): k/v up-projection
    from gathered kv latents; causal flash attention (k-major scores, exp on
    ScalarE, ones-matmul denominator, reciprocal + K=1 broadcast matmul for
    the divide).
  output: two AllToAlls (even heads overlap the odd-head attention; odd heads
    overlap the even half of the token-parallel output projection).

Scheduling notes (v2):
  - DMA issue order keeps the first ckv matmul start at ~1.5us (hid/wkva
    interleaved first; wqb prefetch rides the second HWDGE queue).
  - ckv and cq run kc-outer (accumulate all m-blocks per hid chunk) so
    compute starts before the weight streams finish.
  - kv up-projection runs AFTER the q up-projection so the q AllToAlls hide
    behind it.
  - attention is software-pipelined with a 2-chunk lookahead (PV/denominator
    of chunk i issue after the scores of chunk i+2) and per-q-tile
    normalization is deferred by one chunk; diagonal blocks only compute the
    causally visible column range.
  - o_proj weights prefetch during attention; output AllToAll receives issue
    right after each collective.

bf16 matmuls, fp32 PSUM accumulation + softmax statistics, fp32 output.
"""
import sys
from collections import deque
from contextlib import ExitStack

sys.path.insert(0, "/opt/trn_rl_repo")

import numpy as np
import ml_dtypes

import concourse.bacc as bacc
import concourse.mybir as mybir
import concourse.tile as tile
from concourse.bass_utils import run_bass_kernel_spmd

# ---- problem sizes (hardcoded per spec) ----
HID = 2048; H = 16; QLR = 1536; KVLR = 512
DN = 128; DR = 64; DV = 128; DQ = DN + DR
B = 2; S = 2048
THETA = 10000.0; EPS = 1e-6

NCORES = 8
T = B * S              # 4096 flattened tokens
TPC = T // NCORES      # 512 tokens per core
HPC = H // NCORES      # 2 heads per core
P = 128
NHID = HID // P        # 16
NQLR = QLR // P        # 12
CKW = KVLR + DR        # 576
QT_PER_B = S // 512    # 4 q-tiles of 512 per (b,h) unit
KB_PER_B = S // P      # 16 k-chunks of 128 per batch
WKK = HPC * DN         # 256

BF16 = mybir.dt.bfloat16
F32 = mybir.dt.float32
AF = mybir.ActivationFunctionType

_NC_CACHE = None


def _rope_dual(nc, pool, out_bf16, ps, cos_sb, sin_sb, tag):
    """RoPE on a [128, W] psum holding two 64-row head groups; writes bf16."""
    W = 512
    HDR = DR // 2
    rot = pool.tile([P, W], F32, tag=f"{tag}rot", name=f"{tag}rot")
    for g in range(2):
        o = g * DR
        nc.scalar.mul(rot[o:o + HDR, :], ps[o + HDR:o + DR, :], -1.0)
        nc.scalar.copy(rot[o + HDR:o + DR, :], ps[o:o + HDR, :])
    t1 = pool.tile([P, W], F32, tag=f"{tag}t1", name=f"{tag}t1")
    nc.vector.tensor_mul(t1[:], ps[:], cos_sb[:])
    nc.vector.tensor_mul(rot[:], rot[:], sin_sb[:])
    nc.vector.tensor_add(out_bf16[:], t1[:], rot[:])


def build_nc():
    nc = bacc.Bacc(None, target_bir_lowering=False, debug=False, num_devices=NCORES)

    # ---- per-core external inputs ----
    hidT = nc.dram_tensor("hidT", [HID, TPC], BF16, kind="ExternalInput")
    wqaT = nc.dram_tensor("wqaT", [HID, QLR], BF16, kind="ExternalInput")
    wkvaT = nc.dram_tensor("wkvaT", [HID, CKW], BF16, kind="ExternalInput")
    wqbT = nc.dram_tensor("wqbT", [QLR, H * DQ], BF16, kind="ExternalInput")
    wkvbkT = nc.dram_tensor("wkvbkT", [KVLR, HPC * DN], BF16, kind="ExternalInput")
    wkvbvT = nc.dram_tensor("wkvbvT", [KVLR, HPC * DV], BF16, kind="ExternalInput")
    woT = nc.dram_tensor("woT", [H * DV, HID], BF16, kind="ExternalInput")
    cosd = nc.dram_tensor("cosd", [P, TPC], F32, kind="ExternalInput")
    sind = nc.dram_tensor("sind", [P, TPC], F32, kind="ExternalInput")
    masks = nc.dram_tensor("masks", [P, 4 * 512], BF16, kind="ExternalInput")
    outT = nc.dram_tensor("outT", [HID, TPC], F32, kind="ExternalOutput")

    RG = [list(range(NCORES))]

    with tile.TileContext(nc) as tc:
        with tc.tile_pool(name="dram", bufs=1, space="DRAM") as dram, \
             tc.tile_pool(name="const", bufs=1) as const:
            latkv_in = dram.tile([CKW, TPC], BF16)
            latkv_all = dram.tile([NCORES * CKW, TPC], BF16, addr_space="Shared")
            qa_in = dram.tile([NCORES * 2 * P, TPC], BF16)   # [pe2|h0n] per pair
            qa_out = dram.tile([NCORES * 2 * P, TPC], BF16)
            qb_in = dram.tile([NCORES * P, TPC], BF16)       # h1n per pair
            qb_out = dram.tile([NCORES * P, TPC], BF16)
            oa_in = dram.tile([NCORES * DV, TPC], BF16)      # even heads out
            oa_out = dram.tile([NCORES * DV, TPC], BF16)
            ob_in = dram.tile([NCORES * DV, TPC], BF16)      # odd heads out
            ob_out = dram.tile([NCORES * DV, TPC], BF16)

            ones_col = const.tile([P, 1], BF16)
            nc.vector.memset(ones_col[:], 1.0)
            ones_row = const.tile([1, P], BF16)
            nc.vector.memset(ones_row[:], 1.0)
            eps_t = const.tile([1, 1], F32)
            nc.vector.memset(eps_t[:], EPS)
            cos_sb = const.tile([P, TPC], F32)
            sin_sb = const.tile([P, TPC], F32)
            mask_sb = const.tile([P, 512], BF16)

            # ============ phase-1 pools (LIFO: p1q outer, p1w innermost) ============
            p1q_stack = ExitStack()
            p1q = p1q_stack.enter_context(tc.tile_pool(name="p1q", bufs=1))
            p1t_stack = ExitStack()
            p1t = p1t_stack.enter_context(tc.tile_pool(name="p1t", bufs=1))
            ps1_stack = ExitStack()
            ps1 = ps1_stack.enter_context(tc.tile_pool(name="ps1", bufs=6, space="PSUM"))
            ps1s = ps1_stack.enter_context(tc.tile_pool(name="ps1s", bufs=1, space="PSUM"))
            ps1b = ps1_stack.enter_context(tc.tile_pool(name="ps1b", bufs=1, space="PSUM"))
            p1w_stack = ExitStack()
            p1w = p1w_stack.enter_context(tc.tile_pool(name="p1w", bufs=1))

            hid_ch = [p1w.tile([P, TPC], BF16, tag=f"hid{kc}", name=f"hid{kc}")
                      for kc in range(NHID)]
            wkva_ch = [p1w.tile([P, CKW], BF16, tag=f"wkva{kc}", name=f"wkva{kc}")
                       for kc in range(NHID)]
            wqa_ch = [p1w.tile([P, QLR], BF16, tag=f"wqa{kc}", name=f"wqa{kc}")
                      for kc in range(NHID)]
            WQB = H * DQ  # 3072
            wqb_ch = [p1q.tile([P, WQB], BF16, tag=f"wqb{kc}", name=f"wqb{kc}")
                      for kc in range(NQLR)]
            cqn_sb = p1q.tile([P, NQLR * TPC], BF16)

            # ---- DMA issue order (sync queue): compute-critical first ----
            nc.sync.dma_start(hid_ch[0][:], hidT.ap()[0:P, :])
            nc.sync.dma_start(wkva_ch[0][:], wkvaT.ap()[0:P, :])
            nc.sync.dma_start(cos_sb[:], cosd.ap()[:])
            nc.sync.dma_start(sin_sb[:], sind.ap()[:])
            for kc in range(1, NHID):
                nc.sync.dma_start(hid_ch[kc][:], hidT.ap()[kc * P:(kc + 1) * P, :])
                nc.sync.dma_start(wkva_ch[kc][:], wkvaT.ap()[kc * P:(kc + 1) * P, :])
            for kc in range(0, NHID, 2):
                nc.sync.dma_start(wqa_ch[kc][:], wqaT.ap()[kc * P:(kc + 1) * P, :])
            # second HWDGE queue (scalar engine): odd wqa chunks + mask
            for kc in range(1, NHID, 2):
                nc.scalar.dma_start(wqa_ch[kc][:], wqaT.ap()[kc * P:(kc + 1) * P, :])
            nc.scalar.dma_start(mask_sb[:], masks.ap()[:, 0:512])

            # ============ ckv joint projection (kc-outer) ============
            ps_ckv = [ps1.tile([P, TPC], F32, tag="proj", name=f"psckv{m}")
                      for m in range(4)]
            ps_pe = ps1.tile([DR, TPC], F32, tag="proj")
            for kc in range(NHID):
                for m in range(4):
                    nc.tensor.matmul(ps_ckv[m][:], wkva_ch[kc][:, m * P:(m + 1) * P],
                                     hid_ch[kc][:],
                                     start=(kc == 0), stop=(kc == NHID - 1),
                                     skip_group_check=True)
                nc.tensor.matmul(ps_pe[:], wkva_ch[kc][:, KVLR:CKW], hid_ch[kc][:],
                                 start=(kc == 0), stop=(kc == NHID - 1),
                                 skip_group_check=True)

            # k_pe rope (shared across heads) -> latkv_in[512:576]
            HDR = DR // 2
            rot = p1t.tile([DR, TPC], F32, tag="rot")
            nc.scalar.mul(rot[0:HDR, :], ps_pe[HDR:DR, :], -1.0)
            nc.scalar.copy(rot[HDR:DR, :], ps_pe[0:HDR, :])
            t1 = p1t.tile([DR, TPC], F32, tag="t1")
            nc.vector.tensor_mul(t1[:], ps_pe[:], cos_sb[0:DR, :])
            nc.vector.tensor_mul(rot[:], rot[:], sin_sb[0:DR, :])
            pe_out = p1t.tile([DR, TPC], BF16, tag="peo")
            nc.vector.tensor_add(pe_out[:], t1[:], rot[:])
            nc.sync.dma_start(latkv_in[KVLR:CKW, :], pe_out[:])

            # ckv copies + squares (scalar/vector; run during cq passA below)
            ckv_bf = p1t.tile([P, 4 * TPC], BF16)
            sq_kv = [p1t.tile([P, TPC], BF16, tag="sq", name=f"sqkv{m}", bufs=6)
                     for m in range(4)]
            for m in range(4):
                nc.scalar.copy(ckv_bf[:, m * TPC:(m + 1) * TPC], ps_ckv[m][:])
                nc.vector.tensor_mul(sq_kv[m][:], ckv_bf[:, m * TPC:(m + 1) * TPC],
                                     ckv_bf[:, m * TPC:(m + 1) * TPC])
            for kc in range(NQLR):
                nc.scalar.dma_start(wqb_ch[kc][:], wqbT.ap()[kc * P:(kc + 1) * P, :])

            # ============ cq pass A (m=0..5, kc-outer) ============
            def cq_block(mlist, kc):
                for mi, m in enumerate(mlist):
                    nc.tensor.matmul(ps_cq[mi][:], wqa_ch[kc][:, m * P:(m + 1) * P],
                                     hid_ch[kc][:],
                                     start=(kc == 0), stop=(kc == NHID - 1),
                                     skip_group_check=True)

            mlistA = list(range(6))
            ps_cq = [ps1.tile([P, TPC], F32, tag="proj", name=f"pscqa{m}")
                     for m in range(6)]
            cq_block(mlistA, 0)
            cq_block(mlistA, 1)
            # ssq_kv accumulation (tensor; deps on sq_kv fall in this window)
            ssq_kv = ps1s.tile([1, TPC], F32, tag="ssq")
            for m in range(4):
                nc.tensor.matmul(ssq_kv[:], ones_col[:], sq_kv[m][:],
                                 start=(m == 0), stop=(m == 3),
                                 skip_group_check=True)
            cq_block(mlistA, 2)
            # kv rmsnorm chain + broadcast
            kv_norm = p1t.tile([1, TPC], F32, tag="nrm")
            nc.scalar.activation(kv_norm[:], ssq_kv[:], AF.Sqrt, bias=eps_t[:],
                                 scale=1.0 / KVLR)
            rn_kv = p1t.tile([1, TPC], F32, tag="rn")
            nc.vector.reciprocal(rn_kv[:], kv_norm[:])
            rn_kv_bf = p1t.tile([1, TPC], BF16, tag="rnb")
            nc.scalar.copy(rn_kv_bf[:], rn_kv[:])
            bkv = ps1b.tile([P, TPC], F32, tag="bc")
            nc.tensor.matmul(bkv[:], ones_row[:], rn_kv_bf[:], start=True, stop=True,
                             skip_group_check=True)
            for m in range(4):
                lat_sb = p1t.tile([P, TPC], BF16, tag="lat", bufs=2)
                nc.vector.tensor_mul(lat_sb[:], ckv_bf[:, m * TPC:(m + 1) * TPC], bkv[:])
                nc.sync.dma_start(latkv_in[m * P:(m + 1) * P, :], lat_sb[:])
            nc.gpsimd.collective_compute(
                "AllGather", mybir.AluOpType.bypass, replica_groups=RG,
                ins=[latkv_in.opt()], outs=[latkv_all.opt()])
            for kc in range(3, NHID):
                cq_block(mlistA, kc)
            # copies + squares for pass A blocks (run during pass B)
            sq_q = {}
            for mi, m in enumerate(mlistA):
                sq_q[m] = p1t.tile([P, TPC], BF16, tag="sq", name=f"sqq{m}", bufs=6)
                nc.scalar.copy(cqn_sb[:, m * TPC:(m + 1) * TPC], ps_cq[mi][:])
                nc.vector.tensor_mul(sq_q[m][:], cqn_sb[:, m * TPC:(m + 1) * TPC],
                                     cqn_sb[:, m * TPC:(m + 1) * TPC])

            # ============ cq pass B (m=6..11, m-outer: per-block norm pipelining) ============
            mlistB = list(range(6, 12))
            ssq_q = ps1s.tile([1, TPC], F32, tag="ssq")
            for m in mlistA:
                nc.tensor.matmul(ssq_q[:], ones_col[:], sq_q[m][:],
                                 start=(m == 0), stop=False,
                                 skip_group_check=True)
            for m in mlistB:
                ps = ps1.tile([P, TPC], F32, tag="proj", name=f"pscqb{m}")
                for kc in range(NHID):
                    nc.tensor.matmul(ps[:], wqa_ch[kc][:, m * P:(m + 1) * P],
                                     hid_ch[kc][:],
                                     start=(kc == 0), stop=(kc == NHID - 1),
                                     skip_group_check=True)
                sq_q[m] = p1t.tile([P, TPC], BF16, tag="sq", name=f"sqqb{m}", bufs=6)
                nc.scalar.copy(cqn_sb[:, m * TPC:(m + 1) * TPC], ps[:])
                nc.vector.tensor_mul(sq_q[m][:], cqn_sb[:, m * TPC:(m + 1) * TPC],
                                     cqn_sb[:, m * TPC:(m + 1) * TPC])
                nc.tensor.matmul(ssq_q[:], ones_col[:], sq_q[m][:],
                                 start=False, stop=(m == 11),
                                 skip_group_check=True)
            sq_norm = p1t.tile([1, TPC], F32, tag="nrm")
            nc.scalar.activation(sq_norm[:], ssq_q[:], AF.Sqrt, bias=eps_t[:],
                                 scale=1.0 / QLR)
            rn_q = p1t.tile([1, TPC], F32, tag="rn")
            nc.vector.reciprocal(rn_q[:], sq_norm[:])
            rn_q_bf = p1t.tile([1, TPC], BF16, tag="rnb")
            nc.scalar.copy(rn_q_bf[:], rn_q[:])
            bq = ps1b.tile([P, TPC], F32, tag="bc")
            nc.tensor.matmul(bq[:], ones_row[:], rn_q_bf[:], start=True, stop=True,
                             skip_group_check=True)
            for m in range(NQLR):
                nc.vector.tensor_mul(cqn_sb[:, m * TPC:(m + 1) * TPC],
                                     cqn_sb[:, m * TPC:(m + 1) * TPC], bq[:])
            p1w_stack.close()  # free hid/wkva/wqa SBUF
            p1t_stack.close()

            # right-side pools: attention-lifetime tensors + kv-up receive
            # buffers (fresh addresses -> receives run as soon as the
            # AllGather lands, paced to 2-3 tiles in flight)
            att_a_stack = ExitStack()
            att_a = att_a_stack.enter_context(
                tc.tile_pool(name="att_a", bufs=1, side="right"))
            knope = att_a.tile([P, 2 * T], BF16)
            kpe2 = att_a.tile([P, T], BF16)    # k_pe duplicated rows
            v_sb = att_a.tile([P, (T // P) * WKK], BF16)
            qnope = att_a.tile([P, 2 * T], BF16)
            qpe = att_a.tile([P, T], BF16)     # rows 0-63 h0, 64-127 h1
            p2a_stack = ExitStack()
            p2a = p2a_stack.enter_context(
                tc.tile_pool(name="p2a", bufs=3, side="right"))
            wkk_sb = p2a.tile([P, 4 * WKK], BF16, tag="wkk", bufs=1)
            wkv_sb = p2a.tile([P, 4 * WKK], BF16, tag="wkv", bufs=1)
            for kc in range(4):
                nc.sync.dma_start(wkk_sb[:, kc * WKK:(kc + 1) * WKK],
                                  wkvbkT.ap()[kc * P:(kc + 1) * P, :])
                nc.sync.dma_start(wkv_sb[:, kc * WKK:(kc + 1) * WKK],
                                  wkvbvT.ap()[kc * P:(kc + 1) * P, :])
            ckv_js = []
            def recv_ckv(j, engine):
                basek = j * CKW
                ckv_j = p2a.tile([P, 4 * TPC], BF16, tag="ckvj", name="ckv_j")
                for r in range(4):
                    engine.dma_start(ckv_j[:, r * TPC:(r + 1) * TPC],
                                     latkv_all[basek + r * P: basek + (r + 1) * P, :])
                engine.dma_start(kpe2[0:DR, j * TPC:(j + 1) * TPC],
                                 latkv_all[basek + KVLR: basek + CKW, :])
                engine.dma_start(kpe2[DR:P, j * TPC:(j + 1) * TPC],
                                 latkv_all[basek + KVLR: basek + CKW, :])
                ckv_js.append(ckv_j)
            recv_ckv(0, nc.sync)
            recv_ckv(1, nc.sync)

            # ============ q up-projection for ALL heads ============
            with tc.tile_pool(name="p1qt", bufs=3) as p1qt:
                for mb in range(16):
                    ps = ps1.tile([P, TPC], F32, tag="proj")
                    for kc in range(NQLR):
                        nc.tensor.matmul(ps[:], wqb_ch[kc][:, mb * P:(mb + 1) * P],
                                         cqn_sb[:, kc * TPC:(kc + 1) * TPC],
                                         start=(kc == 0), stop=(kc == NQLR - 1),
                                         skip_group_check=True)
                    qo = p1qt.tile([P, TPC], BF16, tag="qo")
                    if mb % 2 == 0:  # pe2 block -> rope
                        _rope_dual(nc, p1qt, qo, ps, cos_sb, sin_sb, "q")
                    else:
                        nc.scalar.copy(qo[:], ps[:])
                    nc.sync.dma_start(qa_in[mb * P:(mb + 1) * P, :], qo[:])
                nc.gpsimd.collective_compute(
                    "AllToAll", mybir.AluOpType.bypass, replica_groups=RG,
                    ins=[qa_in.opt()], outs=[qa_out.opt()])
                for mb in range(8):
                    ps = ps1.tile([P, TPC], F32, tag="proj")
                    for kc in range(NQLR):
                        nc.tensor.matmul(ps[:], wqb_ch[kc][:, (16 + mb) * P:(17 + mb) * P],
                                         cqn_sb[:, kc * TPC:(kc + 1) * TPC],
                                         start=(kc == 0), stop=(kc == NQLR - 1),
                                         skip_group_check=True)
                    qo = p1qt.tile([P, TPC], BF16, tag="qo")
                    nc.scalar.copy(qo[:], ps[:])
                    nc.sync.dma_start(qb_in[mb * P:(mb + 1) * P, :], qo[:])
                nc.gpsimd.collective_compute(
                    "AllToAll", mybir.AluOpType.bypass, replica_groups=RG,
                    ins=[qb_in.opt()], outs=[qb_out.opt()])
            ps1_stack.close()
            p1q_stack.close()

            # q receives (queue slot: after the qb stores; before paced ckv)
            for i in range(NCORES):
                nc.sync.dma_start(qpe[:, i * TPC:(i + 1) * TPC],
                                  qa_out[i * 2 * P: i * 2 * P + P, :])
                nc.sync.dma_start(qnope[:, i * TPC:(i + 1) * TPC],
                                  qa_out[i * 2 * P + P: (i + 1) * 2 * P, :])
            for i in range(NCORES):
                nc.sync.dma_start(qnope[:, T + i * TPC: T + (i + 1) * TPC],
                                  qb_out[i * P:(i + 1) * P, :])

            # ===== phase 2: k/v up-projection (overlaps the q AllToAlls) =====
            with tc.tile_pool(name="ps2", bufs=4, space="PSUM") as ps2:
                for j in range(NCORES):
                    if j + 2 < NCORES:
                        recv_ckv(j + 2, nc.scalar)  # 2 tiles in flight
                    ckv_j = ckv_js[j]
                    for m in range(HPC):
                        ps = ps2.tile([P, TPC], F32, tag="proj")
                        for kc in range(4):
                            nc.tensor.matmul(
                                ps[:], wkk_sb[:, kc * WKK + m * P: kc * WKK + (m + 1) * P],
                                ckv_j[:, kc * TPC:(kc + 1) * TPC],
                                start=(kc == 0), stop=(kc == 3))
                        nc.scalar.copy(knope[:, m * T + j * TPC: m * T + (j + 1) * TPC], ps[:])
                    for tb in range(TPC // P):
                        ps = ps2.tile([P, WKK], F32, tag="projv")
                        for kc in range(4):
                            nc.tensor.matmul(
                                ps[:], ckv_j[:, kc * TPC + tb * P: kc * TPC + (tb + 1) * P],
                                wkv_sb[:, kc * WKK:(kc + 1) * WKK],
                                start=(kc == 0), stop=(kc == 3))
                        jb = j * (TPC // P) + tb
                        nc.scalar.copy(v_sb[:, jb * WKK:(jb + 1) * WKK], ps[:])
            p2a_stack.close()
            # o_proj weight tiles; transfers are spread across attention
            # (one 512KB block per q-tile normalize) to stay under the
            # DMA-activity power brake.
            p3w_stack = ExitStack()
            p3w = p3w_stack.enter_context(
                tc.tile_pool(name="p3w", bufs=1, side="right"))
            woe_sb = p3w.tile([P, NCORES * HID], BF16)
            woo_sb = p3w.tile([P, NCORES * HID], BF16)
            oe_sb = p3w.tile([P, NCORES * TPC], BF16)
            oo_sb = p3w.tile([P, NCORES * TPC], BF16)
            part_sb = p3w.tile([P, NHID * TPC], F32)

            # ============ attention: software-pipelined, causal-trimmed ============
            with tc.tile_pool(name="att_t", bufs=1) as att_t, \
                 tc.tile_pool(name="ps_s", bufs=4, space="PSUM") as ps_s_pool, \
                 tc.tile_pool(name="ps_o", bufs=2, space="PSUM") as ps_o_pool, \
                 tc.tile_pool(name="ps_d", bufs=2, space="PSUM") as ps_d_pool:

                tasks = []
                for u in range(4):          # hl-major: (hl, bb)
                    hl, bb = u // 2, u % 2
                    for qt in range(QT_PER_B):
                        nkc = 4 * (qt + 1)
                        for kc in range(nkc):
                            tasks.append((u, hl, bb, qt, kc, kc == nkc - 1))

                state = {}  # per (u,qt): dict(ps_o, ps_d, ex tiles keyed by kc)
                ex_of = {}  # task idx -> (ex tile, m0)
                norm_pending = None

                def issue_scores(i, t):
                    u, hl, bb, qt, kc, _ = t
                    qoff = bb * S + qt * 512
                    koff = bb * S + kc * P
                    mi = kc - 4 * qt
                    m0 = mi * P if mi >= 0 else 0
                    ps_sc = ps_s_pool.tile([P, 512], F32, tag="pss")
                    nc.tensor.matmul(
                        ps_sc[:, m0:512], knope[:, hl * T + koff: hl * T + koff + P],
                        qnope[:, hl * T + qoff + m0: hl * T + qoff + 512],
                        start=True, stop=False, skip_group_check=True)
                    nc.tensor.matmul(
                        ps_sc[:, m0:512], kpe2[hl * DR: hl * DR + DR, koff: koff + P],
                        qpe[hl * DR: hl * DR + DR, qoff + m0: qoff + 512],
                        start=False, stop=True, skip_group_check=True)
                    ex = att_t.tile([P, 512], BF16, tag="ex", bufs=5)
                    nc.scalar.activation(ex[:, m0:512], ps_sc[:, m0:512], AF.Exp)
                    if mi >= 0:
                        nc.vector.tensor_mul(ex[:, m0:m0 + P], ex[:, m0:m0 + P],
                                             mask_sb[:, 0:P])
                    ex_of[i] = (ex, m0)

                def issue_pv(i, t):
                    """PV + denominator for task i; returns norm info if qt closed."""
                    u, hl, bb, qt, kc, is_last = t
                    ex, m0 = ex_of.pop(i)
                    key = (u, qt)
                    if key not in state:
                        state[key] = {
                            "ps_o": ps_o_pool.tile([P, 512], F32, tag="pso",
                                                   name="pso"),
                            "ps_d": ps_d_pool.tile([1, 512], F32, tag="psd",
                                                   name="psd"),
                        }
                    st = state[key]
                    jb = bb * KB_PER_B + kc
                    nc.tensor.matmul(
                        st["ps_o"][:, m0:512],
                        v_sb[:, jb * WKK + hl * DV: jb * WKK + (hl + 1) * DV],
                        ex[:, m0:512], start=(kc == 0), stop=is_last,
                        skip_group_check=True)
                    nc.tensor.matmul(
                        st["ps_d"][:, m0:512], ones_col[:], ex[:, m0:512],
                        start=(kc == 0), stop=is_last, skip_group_check=True)
                    if is_last:
                        recip = att_t.tile([1, 512], F32, tag="rcp", bufs=2)
                        nc.vector.reciprocal_approx_fast(recip[:], st["ps_d"][:])
                        recip_bf = att_t.tile([1, 512], BF16, tag="rcpb", bufs=2)
                        nc.scalar.copy(recip_bf[:], recip[:])
                        return (u, hl, bb, qt, st, recip_bf)
                    return None

                norm_count = [0]

                def issue_norm(info):
                    u, hl, bb, qt, st, recip = info
                    bc = ps_s_pool.tile([P, 512], F32, tag="pss", name="bc")
                    nc.tensor.matmul(bc[:], ones_row[:], recip[:], start=True,
                                     stop=True, skip_group_check=True)
                    ou = att_t.tile([P, 512], F32, tag="ou", bufs=2)
                    nc.scalar.copy(ou[:], st["ps_o"][:])
                    on = att_t.tile([P, 512], BF16, tag="on", bufs=2)
                    nc.vector.tensor_mul(on[:], ou[:], bc[:])
                    blk = bb * QT_PER_B + qt
                    tgt = oa_in if hl == 0 else ob_in
                    nc.sync.dma_start(tgt[blk * DV:(blk + 1) * DV, :], on[:])
                    del state[(u, qt)]
                    # spread o_proj weight prefetch: one 512KB block per norm
                    e = norm_count[0]
                    norm_count[0] += 1
                    if e < 8:
                        nc.sync.dma_start(woe_sb[:, e * HID:(e + 1) * HID],
                                          woT.ap()[(2 * e) * P:(2 * e + 1) * P, :])
                    else:
                        eo = e - 8
                        nc.sync.dma_start(woo_sb[:, eo * HID:(eo + 1) * HID],
                                          woT.ap()[(2 * eo + 1) * P:(2 * eo + 2) * P, :])
                    if u == 1 and qt == QT_PER_B - 1:
                        # even heads complete -> ship while odd attention runs
                        nc.gpsimd.collective_compute(
                            "AllToAll", mybir.AluOpType.bypass, replica_groups=RG,
                            ins=[oa_in.opt()], outs=[oa_out.opt()])
                        for i2 in range(NCORES):
                            nc.sync.dma_start(oe_sb[:, i2 * TPC:(i2 + 1) * TPC],
                                              oa_out[i2 * P:(i2 + 1) * P, :])

                LOOKAHEAD = 3
                pend = deque()
                for i, t in enumerate(tasks):
                    issue_scores(i, t)
                    if norm_pending is not None:
                        issue_norm(norm_pending)
                        norm_pending = None
                    if len(pend) == LOOKAHEAD:
                        pi, pt = pend.popleft()
                        norm_pending = issue_pv(pi, pt)
                    pend.append((i, t))
                while pend:
                    if norm_pending is not None:
                        issue_norm(norm_pending)
                        norm_pending = None
                    pi, pt = pend.popleft()
                    norm_pending = issue_pv(pi, pt)
                if norm_pending is not None:
                    issue_norm(norm_pending)

            nc.gpsimd.collective_compute(
                "AllToAll", mybir.AluOpType.bypass, replica_groups=RG,
                ins=[ob_in.opt()], outs=[ob_out.opt()])
            for i in range(NCORES):
                nc.sync.dma_start(oo_sb[:, i * TPC:(i + 1) * TPC],
                                  ob_out[i * P:(i + 1) * P, :])

            # ============ phase 3: o_proj (pass 1 overlaps the ob AllToAll) ============
            with tc.tile_pool(name="p3t", bufs=3) as p3t, \
                 tc.tile_pool(name="ps3", bufs=4, space="PSUM") as ps3:
                for m in range(NHID):
                    ps = ps3.tile([P, TPC], F32, tag="proj")
                    for i in range(NCORES):
                        nc.tensor.matmul(
                            ps[:], woe_sb[:, i * HID + m * P: i * HID + (m + 1) * P],
                            oe_sb[:, i * TPC:(i + 1) * TPC],
                            start=(i == 0), stop=(i == NCORES - 1))
                    nc.scalar.copy(part_sb[:, m * TPC:(m + 1) * TPC], ps[:])
                for m in range(NHID):
                    ps = ps3.tile([P, TPC], F32, tag="proj")
                    for i in range(NCORES):
                        nc.tensor.matmul(
                            ps[:], woo_sb[:, i * HID + m * P: i * HID + (m + 1) * P],
                            oo_sb[:, i * TPC:(i + 1) * TPC],
                            start=(i == 0), stop=(i == NCORES - 1))
                    ot = p3t.tile([P, TPC], F32, tag="ot")
                    nc.vector.tensor_add(ot[:], ps[:], part_sb[:, m * TPC:(m + 1) * TPC])
                    nc.sync.dma_start(outT.ap()[m * P:(m + 1) * P, :], ot[:])
            p3w_stack.close()
            att_a_stack.close()
    nc.finalize()
    return nc


def _bf16(x):
    return np.ascontiguousarray(x.astype(ml_dtypes.bfloat16))


def _rope_tables():
    inv_freq = 1.0 / (THETA ** (np.arange(0, DR, 2, dtype=np.float64) / DR))
    t = np.arange(S, dtype=np.float64)
    freqs = np.outer(t, inv_freq)
    emb = np.concatenate((freqs, freqs), axis=-1)
    return np.cos(emb).astype(np.float32), np.sin(emb).astype(np.float32)


def prepare_inputs(hidden_states, w_qa, q_a_ln_w, w_qb, w_kva, kv_a_ln_w, w_kvb, w_o):
    hidden_states = np.asarray(hidden_states, dtype=np.float32)
    w_qa = np.asarray(w_qa, dtype=np.float32)
    q_a_ln_w = np.asarray(q_a_ln_w, dtype=np.float32)
    w_qb = np.asarray(w_qb, dtype=np.float32)
    w_kva = np.asarray(w_kva, dtype=np.float32)
    kv_a_ln_w = np.asarray(kv_a_ln_w, dtype=np.float32)
    w_kvb = np.asarray(w_kvb, dtype=np.float32)
    w_o = np.asarray(w_o, dtype=np.float32)

    flat = hidden_states.reshape(T, HID)
    cos, sin = _rope_tables()          # [S, DR]
    scale = DQ ** -0.5

    pos = np.arange(T) % S
    cos_d = cos[pos].T                 # [DR, T]
    sin_d = sin[pos].T

    kp = np.arange(P)[:, None]
    qf = np.arange(512)[None, :]
    masks = _bf16(np.concatenate(
        [(qf >= kp + P * p).astype(np.float32) for p in range(4)], axis=1))

    w_qb_eff = (w_qb * q_a_ln_w[None, :]) * scale       # [H*DQ, QLR]
    w_kvb_eff = w_kvb * kv_a_ln_w[None, :]              # [H*(DN+DV), KVLR]

    # w_qb rows permuted: block A = per pair j [h0 pe | h1 pe | h0 nope],
    # block B = per pair j [h1 nope]
    rows = []
    for j in range(NCORES):
        h0, h1 = 2 * j, 2 * j + 1
        rows.append(w_qb_eff[h0 * DQ + DN: h0 * DQ + DQ])   # h0 pe (64)
        rows.append(w_qb_eff[h1 * DQ + DN: h1 * DQ + DQ])   # h1 pe (64)
        rows.append(w_qb_eff[h0 * DQ: h0 * DQ + DN])        # h0 nope (128)
    for j in range(NCORES):
        h1 = 2 * j + 1
        rows.append(w_qb_eff[h1 * DQ: h1 * DQ + DN])        # h1 nope (128)
    wqbT_full = _bf16(np.concatenate(rows, axis=0).T)       # [QLR, 3072]

    wqaT = _bf16(w_qa.T)
    wkvaT = _bf16(w_kva.T)
    woT = _bf16(w_o.T)

    in_maps = []
    for c in range(NCORES):
        heads = [HPC * c + h for h in range(HPC)]
        krows = [w_kvb_eff[h * (DN + DV): h * (DN + DV) + DN] for h in heads]
        wkvbkT_c = _bf16(np.concatenate(krows, axis=0).T)
        vrows = [w_kvb_eff[h * (DN + DV) + DN: (h + 1) * (DN + DV)] for h in heads]
        wkvbvT_c = _bf16(np.concatenate(vrows, axis=0).T)

        tok0 = c * TPC
        cosl = cos_d[:, tok0:tok0 + TPC]
        sinl = sin_d[:, tok0:tok0 + TPC]
        in_maps.append({
            "hidT": _bf16(flat[tok0:tok0 + TPC].T),
            "wqaT": wqaT, "wkvaT": wkvaT,
            "wqbT": wqbT_full, "wkvbkT": wkvbkT_c, "wkvbvT": wkvbvT_c,
            "woT": woT,
            "cosd": np.ascontiguousarray(np.concatenate([cosl, cosl], axis=0)),
            "sind": np.ascontiguousarray(np.concatenate([sinl, sinl], axis=0)),
            "masks": masks,
        })
    return in_maps


def kernel(hidden_states, w_qa, q_a_ln_w, w_qb, w_kva, kv_a_ln_w, w_kvb, w_o,
           _trace=False):
    global _NC_CACHE
    if _NC_CACHE is None:
        _NC_CACHE = build_nc()
    nc = _NC_CACHE
    in_maps = prepare_inputs(hidden_states, w_qa, q_a_ln_w, w_qb, w_kva,
                             kv_a_ln_w, w_kvb, w_o)
    res = run_bass_kernel_spmd(nc, in_maps, core_ids=list(range(NCORES)),
                               trace=_trace)
    out = np.empty((T, HID), dtype=np.float32)
    for c in range(NCORES):
        out[c * TPC:(c + 1) * TPC] = res.results[c]["outT"].T
    if _trace:
        kernel._last_result = res
    return out.reshape(B, S, HID)


# revision 22
# speedup vs baseline: 1.2867x; 1.0530x over previous
"""MLA (DeepSeek-style) attention block on 8 Trainium2 NeuronCores.

Sharding:
  phase 1 (token-parallel, 8 x 512 tokens): LoRA-A down-projections + rmsnorm
    + k_pe rope; small AllGather of the kv latents (576 dims); q up-projection
    for ALL 16 heads on the token side + rope, shipped to head shards via two
    shard-aligned AllToAlls (pe+even-nope first, odd-nope second).
  phase 2 (head-parallel, 2 heads x 2 batches per core): k/v up-projection
    from gathered kv latents; causal flash attention (k-major scores, exp on
    ScalarE, ones-matmul denominator, reciprocal + K=1 broadcast matmul for
    the divide).
  output: two AllToAlls (even heads overlap the odd-head attention; odd heads
    overlap the even half of the token-parallel output projection).

Scheduling notes (v2):
  - DMA issue order keeps the first ckv matmul start at ~1.5us (hid/wkva
    interleaved first; wqb prefetch rides the second HWDGE queue).
  - ckv and cq run kc-outer (accumulate all m-blocks per hid chunk) so
    compute starts before the weight streams finish.
  - kv up-projection runs AFTER the q up-projection so the q AllToAlls hide
    behind it.
  - attention is software-pipelined with a 2-chunk lookahead (PV/denominator
    of chunk i issue after the scores of chunk i+2) and per-q-tile
    normalization is deferred by one chunk; diagonal blocks only compute the
    causally visible column range.
  - o_proj weights prefetch during attention; output AllToAll receives issue
    right after each collective.

bf16 matmuls, fp32 PSUM accumulation + softmax statistics, fp32 output.
"""
import sys
from collections import deque
from contextlib import ExitStack

sys.path.insert(0, "/opt/trn_rl_repo")

import numpy as np
import ml_dtypes

import concourse.bacc as bacc
import concourse.mybir as mybir
import concourse.tile as tile
from concourse.bass_utils import run_bass_kernel_spmd

# ---- problem sizes (hardcoded per spec) ----
HID = 2048; H = 16; QLR = 1536; KVLR = 512
DN = 128; DR = 64; DV = 128; DQ = DN + DR
B = 2; S = 2048
THETA = 10000.0; EPS = 1e-6

NCORES = 8
T = B * S              # 4096 flattened tokens
TPC = T // NCORES      # 512 tokens per core
HPC = H // NCORES      # 2 heads per core
P = 128
NHID = HID // P        # 16
NQLR = QLR // P        # 12
CKW = KVLR + DR        # 576
QT_PER_B = S // 512    # 4 q-tiles of 512 per (b,h) unit
KB_PER_B = S // P      # 16 k-chunks of 128 per batch
WKK = HPC * DN         # 256

BF16 = mybir.dt.bfloat16
F32 = mybir.dt.float32
AF = mybir.ActivationFunctionType

_NC_CACHE = None


def _rope_dual(nc, pool, out_bf16, ps, cos_sb, sin_sb, tag):
    """RoPE on a [128, W] psum holding two 64-row head groups; writes bf16."""
    W = 512
    HDR = DR // 2
    rot = pool.tile([P, W], F32, tag=f"{tag}rot", name=f"{tag}rot")
    for g in range(2):
        o = g * DR
        nc.scalar.mul(rot[o:o + HDR, :], ps[o + HDR:o + DR, :], -1.0)
        nc.scalar.copy(rot[o + HDR:o + DR, :], ps[o:o + HDR, :])
    t1 = pool.tile([P, W], F32, tag=f"{tag}t1", name=f"{tag}t1")
    nc.vector.tensor_mul(t1[:], ps[:], cos_sb[:])
    nc.vector.tensor_mul(rot[:], rot[:], sin_sb[:])
    nc.vector.tensor_add(out_bf16[:], t1[:], rot[:])


def build_nc():
    nc = bacc.Bacc(None, target_bir_lowering=False, debug=False, num_devices=NCORES)

    # ---- per-core external inputs ----
    hidT = nc.dram_tensor("hidT", [HID, TPC], BF16, kind="ExternalInput")
    wqaT = nc.dram_tensor("wqaT", [HID, QLR], BF16, kind="ExternalInput")
    wkvaT = nc.dram_tensor("wkvaT", [HID, CKW], BF16, kind="ExternalInput")
    wqbT = nc.dram_tensor("wqbT", [QLR, H * DQ], BF16, kind="ExternalInput")
    wkvbkT = nc.dram_tensor("wkvbkT", [KVLR, HPC * DN], BF16, kind="ExternalInput")
    wkvbvT = nc.dram_tensor("wkvbvT", [KVLR, HPC * DV], BF16, kind="ExternalInput")
    woT = nc.dram_tensor("woT", [H * DV, HID], BF16, kind="ExternalInput")
    cosd = nc.dram_tensor("cosd", [P, TPC], F32, kind="ExternalInput")
    sind = nc.dram_tensor("sind", [P, TPC], F32, kind="ExternalInput")
    masks = nc.dram_tensor("masks", [P, 4 * 512], BF16, kind="ExternalInput")
    outT = nc.dram_tensor("outT", [HID, TPC], F32, kind="ExternalOutput")

    RG = [list(range(NCORES))]

    with tile.TileContext(nc) as tc:
        with tc.tile_pool(name="dram", bufs=1, space="DRAM") as dram, \
             tc.tile_pool(name="const", bufs=1) as const:
            latkv_in = dram.tile([CKW, TPC], BF16)
            latkv_all = dram.tile([NCORES * CKW, TPC], BF16, addr_space="Shared")
            qa_in = dram.tile([NCORES * 2 * P, TPC], BF16)   # [pe2|h0n] per pair
            qa_out = dram.tile([NCORES * 2 * P, TPC], BF16)
            qb_in = dram.tile([NCORES * P, TPC], BF16)       # h1n per pair
            qb_out = dram.tile([NCORES * P, TPC], BF16)
            oa_in = dram.tile([NCORES * DV, TPC], BF16)      # even heads out
            oa_out = dram.tile([NCORES * DV, TPC], BF16)
            ob_in = dram.tile([NCORES * DV, TPC], BF16)      # odd heads out
            ob_out = dram.tile([NCORES * DV, TPC], BF16)

            ones_col = const.tile([P, 1], BF16)
            nc.vector.memset(ones_col[:], 1.0)
            ones_row = const.tile([1, P], BF16)
            nc.vector.memset(ones_row[:], 1.0)
            eps_t = const.tile([1, 1], F32)
            nc.vector.memset(eps_t[:], EPS)
            cos_sb = const.tile([P, TPC], F32)
            sin_sb = const.tile([P, TPC], F32)
            mask_sb = const.tile([P, 512], BF16)
            neg_sb = const.tile([P, P], BF16)   # (mask-1)*1e30: additive causal mask

            # ============ phase-1 pools (LIFO: p1q outer, p1w innermost) ============
            p1q_stack = ExitStack()
            p1q = p1q_stack.enter_context(tc.tile_pool(name="p1q", bufs=1))
            p1t_stack = ExitStack()
            p1t = p1t_stack.enter_context(tc.tile_pool(name="p1t", bufs=1))
            ps1_stack = ExitStack()
            ps1 = ps1_stack.enter_context(tc.tile_pool(name="ps1", bufs=6, space="PSUM"))
            ps1s = ps1_stack.enter_context(tc.tile_pool(name="ps1s", bufs=1, space="PSUM"))
            ps1b = ps1_stack.enter_context(tc.tile_pool(name="ps1b", bufs=1, space="PSUM"))
            p1w_stack = ExitStack()
            p1w = p1w_stack.enter_context(tc.tile_pool(name="p1w", bufs=1))

            hid_ch = [p1w.tile([P, TPC], BF16, tag=f"hid{kc}", name=f"hid{kc}")
                      for kc in range(NHID)]
            wkva_ch = [p1w.tile([P, CKW], BF16, tag=f"wkva{kc}", name=f"wkva{kc}")
                       for kc in range(NHID)]
            wqa_ch = [p1w.tile([P, QLR], BF16, tag=f"wqa{kc}", name=f"wqa{kc}")
                      for kc in range(NHID)]
            WQB = H * DQ  # 3072
            wqb_ch = [p1q.tile([P, WQB], BF16, tag=f"wqb{kc}", name=f"wqb{kc}")
                      for kc in range(NQLR)]
            cqn_sb = p1q.tile([P, NQLR * TPC], BF16)

            # ---- DMA issue order: interleave the two HWDGE queues ----
            nc.sync.dma_start(hid_ch[0][:], hidT.ap()[0:P, :])
            nc.sync.dma_start(wkva_ch[0][:], wkvaT.ap()[0:P, :])
            nc.scalar.dma_start(hid_ch[1][:], hidT.ap()[P:2 * P, :])
            nc.scalar.dma_start(wkva_ch[1][:], wkvaT.ap()[P:2 * P, :])
            nc.sync.dma_start(cos_sb[:], cosd.ap()[:])
            nc.sync.dma_start(sin_sb[:], sind.ap()[:])
            nc.scalar.dma_start(mask_sb[:], masks.ap()[:, 0:512])
            for kc in range(2, NHID):
                eng = nc.sync if kc % 2 == 0 else nc.scalar
                eng.dma_start(hid_ch[kc][:], hidT.ap()[kc * P:(kc + 1) * P, :])
                eng.dma_start(wkva_ch[kc][:], wkvaT.ap()[kc * P:(kc + 1) * P, :])
            for kc in range(NHID):
                eng = nc.sync if kc % 2 == 0 else nc.scalar
                eng.dma_start(wqa_ch[kc][:], wqaT.ap()[kc * P:(kc + 1) * P, :])

            # ============ ckv joint projection (kc-outer) ============
            nc.scalar.activation(neg_sb[:], mask_sb[:, 0:P], AF.Copy,
                                 bias=-1e30, scale=1e30)

            ps_ckv = [ps1.tile([P, TPC], F32, tag="proj", name=f"psckv{m}")
                      for m in range(4)]
            ps_pe = ps1.tile([DR, TPC], F32, tag="proj")
            for kc in range(NHID):
                for m in range(4):
                    nc.tensor.matmul(ps_ckv[m][:], wkva_ch[kc][:, m * P:(m + 1) * P],
                                     hid_ch[kc][:],
                                     start=(kc == 0), stop=(kc == NHID - 1),
                                     skip_group_check=True)
                nc.tensor.matmul(ps_pe[:], wkva_ch[kc][:, KVLR:CKW], hid_ch[kc][:],
                                 start=(kc == 0), stop=(kc == NHID - 1),
                                 skip_group_check=True)

            # k_pe rope (shared across heads) -> latkv_in[512:576]
            HDR = DR // 2
            rot = p1t.tile([DR, TPC], F32, tag="rot")
            nc.scalar.mul(rot[0:HDR, :], ps_pe[HDR:DR, :], -1.0)
            nc.scalar.copy(rot[HDR:DR, :], ps_pe[0:HDR, :])
            t1 = p1t.tile([DR, TPC], F32, tag="t1")
            nc.vector.tensor_mul(t1[:], ps_pe[:], cos_sb[0:DR, :])
            nc.vector.tensor_mul(rot[:], rot[:], sin_sb[0:DR, :])
            pe_out = p1t.tile([DR, TPC], BF16, tag="peo")
            nc.vector.tensor_add(pe_out[:], t1[:], rot[:])
            nc.sync.dma_start(latkv_in[KVLR:CKW, :], pe_out[:])

            # ckv copies + squares (scalar/vector; run during cq passA below)
            ckv_bf = p1t.tile([P, 4 * TPC], BF16)
            sq_kv = [p1t.tile([P, TPC], BF16, tag="sq", name=f"sqkv{m}", bufs=6)
                     for m in range(4)]
            for m in range(4):
                nc.scalar.copy(ckv_bf[:, m * TPC:(m + 1) * TPC], ps_ckv[m][:])
                nc.vector.tensor_mul(sq_kv[m][:], ckv_bf[:, m * TPC:(m + 1) * TPC],
                                     ckv_bf[:, m * TPC:(m + 1) * TPC])
            for kc in range(NQLR):
                nc.scalar.dma_start(wqb_ch[kc][:], wqbT.ap()[kc * P:(kc + 1) * P, :])

            # ============ cq pass A (m=0..5, kc-outer) ============
            def cq_block(mlist, kc):
                for mi, m in enumerate(mlist):
                    nc.tensor.matmul(ps_cq[mi][:], wqa_ch[kc][:, m * P:(m + 1) * P],
                                     hid_ch[kc][:],
                                     start=(kc == 0), stop=(kc == NHID - 1),
                                     skip_group_check=True)

            mlistA = list(range(6))
            ps_cq = [ps1.tile([P, TPC], F32, tag="proj", name=f"pscqa{m}")
                     for m in range(6)]
            cq_block(mlistA, 0)
            cq_block(mlistA, 1)
            # ssq_kv accumulation (tensor; deps on sq_kv fall in this window)
            ssq_kv = ps1s.tile([1, TPC], F32, tag="ssq")
            for m in range(4):
                nc.tensor.matmul(ssq_kv[:], ones_col[:], sq_kv[m][:],
                                 start=(m == 0), stop=(m == 3),
                                 skip_group_check=True)
            cq_block(mlistA, 2)
            # kv rmsnorm chain + broadcast
            kv_norm = p1t.tile([1, TPC], F32, tag="nrm")
            nc.scalar.activation(kv_norm[:], ssq_kv[:], AF.Sqrt, bias=eps_t[:],
                                 scale=1.0 / KVLR)
            rn_kv = p1t.tile([1, TPC], F32, tag="rn")
            nc.vector.reciprocal(rn_kv[:], kv_norm[:])
            rn_kv_bf = p1t.tile([1, TPC], BF16, tag="rnb")
            nc.scalar.copy(rn_kv_bf[:], rn_kv[:])
            bkv = ps1b.tile([P, TPC], F32, tag="bc")
            nc.tensor.matmul(bkv[:], ones_row[:], rn_kv_bf[:], start=True, stop=True,
                             skip_group_check=True)
            for m in range(4):
                lat_sb = p1t.tile([P, TPC], BF16, tag="lat", bufs=2)
                nc.vector.tensor_mul(lat_sb[:], ckv_bf[:, m * TPC:(m + 1) * TPC], bkv[:])
                nc.sync.dma_start(latkv_in[m * P:(m + 1) * P, :], lat_sb[:])
            nc.gpsimd.collective_compute(
                "AllGather", mybir.AluOpType.bypass, replica_groups=RG,
                ins=[latkv_in.opt()], outs=[latkv_all.opt()])
            for kc in range(3, NHID):
                cq_block(mlistA, kc)
            # copies + squares for pass A blocks (run during pass B)
            sq_q = {}
            for mi, m in enumerate(mlistA):
                sq_q[m] = p1t.tile([P, TPC], BF16, tag="sq", name=f"sqq{m}", bufs=6)
                nc.scalar.copy(cqn_sb[:, m * TPC:(m + 1) * TPC], ps_cq[mi][:])
                nc.vector.tensor_mul(sq_q[m][:], cqn_sb[:, m * TPC:(m + 1) * TPC],
                                     cqn_sb[:, m * TPC:(m + 1) * TPC])

            # ============ cq pass B (m=6..11, m-outer: per-block norm pipelining) ============
            mlistB = list(range(6, 12))
            ssq_q = ps1s.tile([1, TPC], F32, tag="ssq")
            for m in mlistA:
                nc.tensor.matmul(ssq_q[:], ones_col[:], sq_q[m][:],
                                 start=(m == 0), stop=False,
                                 skip_group_check=True)
            for m in mlistB:
                ps = ps1.tile([P, TPC], F32, tag="proj", name=f"pscqb{m}")
                for kc in range(NHID):
                    nc.tensor.matmul(ps[:], wqa_ch[kc][:, m * P:(m + 1) * P],
                                     hid_ch[kc][:],
                                     start=(kc == 0), stop=(kc == NHID - 1),
                                     skip_group_check=True)
                sq_q[m] = p1t.tile([P, TPC], BF16, tag="sq", name=f"sqqb{m}", bufs=6)
                nc.scalar.copy(cqn_sb[:, m * TPC:(m + 1) * TPC], ps[:])
                nc.vector.tensor_mul(sq_q[m][:], cqn_sb[:, m * TPC:(m + 1) * TPC],
                                     cqn_sb[:, m * TPC:(m + 1) * TPC])
                nc.tensor.matmul(ssq_q[:], ones_col[:], sq_q[m][:],
                                 start=False, stop=(m == 11),
                                 skip_group_check=True)
            sq_norm = p1t.tile([1, TPC], F32, tag="nrm")
            nc.scalar.activation(sq_norm[:], ssq_q[:], AF.Sqrt, bias=eps_t[:],
                                 scale=1.0 / QLR)
            rn_q = p1t.tile([1, TPC], F32, tag="rn")
            nc.vector.reciprocal(rn_q[:], sq_norm[:])
            rn_q_bf = p1t.tile([1, TPC], BF16, tag="rnb")
            nc.scalar.copy(rn_q_bf[:], rn_q[:])
            bq = ps1b.tile([P, TPC], F32, tag="bc")
            nc.tensor.matmul(bq[:], ones_row[:], rn_q_bf[:], start=True, stop=True,
                             skip_group_check=True)
            for m in range(NQLR):
                nc.vector.tensor_mul(cqn_sb[:, m * TPC:(m + 1) * TPC],
                                     cqn_sb[:, m * TPC:(m + 1) * TPC], bq[:])
            p1w_stack.close()  # free hid/wkva/wqa SBUF
            p1t_stack.close()

            # right-side pools: attention-lifetime tensors + kv-up receive
            # buffers (fresh addresses -> receives run as soon as the
            # AllGather lands, paced to 2-3 tiles in flight)
            att_a_stack = ExitStack()
            att_a = att_a_stack.enter_context(
                tc.tile_pool(name="att_a", bufs=1, side="right"))
            knope = att_a.tile([P, 2 * T], BF16)
            kpe2 = att_a.tile([P, T], BF16)    # k_pe duplicated rows
            v_sb = att_a.tile([P, (T // P) * WKK], BF16)
            qnope = att_a.tile([P, 2 * T], BF16)
            qpe = att_a.tile([P, T], BF16)     # rows 0-63 h0, 64-127 h1
            p2a_stack = ExitStack()
            p2a = p2a_stack.enter_context(
                tc.tile_pool(name="p2a", bufs=3, side="right"))
            wkk_sb = p2a.tile([P, 4 * WKK], BF16, tag="wkk", bufs=1)
            wkv_sb = p2a.tile([P, 4 * WKK], BF16, tag="wkv", bufs=1)
            for kc in range(4):
                nc.sync.dma_start(wkk_sb[:, kc * WKK:(kc + 1) * WKK],
                                  wkvbkT.ap()[kc * P:(kc + 1) * P, :])
                nc.sync.dma_start(wkv_sb[:, kc * WKK:(kc + 1) * WKK],
                                  wkvbvT.ap()[kc * P:(kc + 1) * P, :])
            ckv_js = []
            def recv_ckv(j, engine):
                basek = j * CKW
                ckv_j = p2a.tile([P, 4 * TPC], BF16, tag="ckvj", name="ckv_j")
                for r in range(4):
                    engine.dma_start(ckv_j[:, r * TPC:(r + 1) * TPC],
                                     latkv_all[basek + r * P: basek + (r + 1) * P, :])
                engine.dma_start(kpe2[0:DR, j * TPC:(j + 1) * TPC],
                                 latkv_all[basek + KVLR: basek + CKW, :])
                engine.dma_start(kpe2[DR:P, j * TPC:(j + 1) * TPC],
                                 latkv_all[basek + KVLR: basek + CKW, :])
                ckv_js.append(ckv_j)
            recv_ckv(0, nc.sync)
            recv_ckv(1, nc.sync)

            # ============ q up-projection for ALL heads ============
            with tc.tile_pool(name="p1qt", bufs=3) as p1qt:
                for mb in range(16):
                    ps = ps1.tile([P, TPC], F32, tag="proj")
                    for kc in range(NQLR):
                        nc.tensor.matmul(ps[:], wqb_ch[kc][:, mb * P:(mb + 1) * P],
                                         cqn_sb[:, kc * TPC:(kc + 1) * TPC],
                                         start=(kc == 0), stop=(kc == NQLR - 1),
                                         skip_group_check=True)
                    qo = p1qt.tile([P, TPC], BF16, tag="qo")
                    if mb % 2 == 0:  # pe2 block -> rope
                        _rope_dual(nc, p1qt, qo, ps, cos_sb, sin_sb, "q")
                    else:
                        nc.scalar.copy(qo[:], ps[:])
                    nc.sync.dma_start(qa_in[mb * P:(mb + 1) * P, :], qo[:])
                nc.gpsimd.collective_compute(
                    "AllToAll", mybir.AluOpType.bypass, replica_groups=RG,
                    ins=[qa_in.opt()], outs=[qa_out.opt()])
                for mb in range(8):
                    ps = ps1.tile([P, TPC], F32, tag="proj")
                    for kc in range(NQLR):
                        nc.tensor.matmul(ps[:], wqb_ch[kc][:, (16 + mb) * P:(17 + mb) * P],
                                         cqn_sb[:, kc * TPC:(kc + 1) * TPC],
                                         start=(kc == 0), stop=(kc == NQLR - 1),
                                         skip_group_check=True)
                    qo = p1qt.tile([P, TPC], BF16, tag="qo")
                    nc.scalar.copy(qo[:], ps[:])
                    nc.sync.dma_start(qb_in[mb * P:(mb + 1) * P, :], qo[:])
                nc.gpsimd.collective_compute(
                    "AllToAll", mybir.AluOpType.bypass, replica_groups=RG,
                    ins=[qb_in.opt()], outs=[qb_out.opt()])
            ps1_stack.close()
            p1q_stack.close()

            # q receives (queue slot: after the qb stores; before paced ckv)
            for i in range(NCORES):
                nc.sync.dma_start(qpe[:, i * TPC:(i + 1) * TPC],
                                  qa_out[i * 2 * P: i * 2 * P + P, :])
                nc.sync.dma_start(qnope[:, i * TPC:(i + 1) * TPC],
                                  qa_out[i * 2 * P + P: (i + 1) * 2 * P, :])
            for i in range(NCORES):
                nc.sync.dma_start(qnope[:, T + i * TPC: T + (i + 1) * TPC],
                                  qb_out[i * P:(i + 1) * P, :])

            # ===== phase 2: k/v up-projection (overlaps the q AllToAlls) =====
            with tc.tile_pool(name="ps2", bufs=4, space="PSUM") as ps2:
                for j in range(NCORES):
                    if j + 2 < NCORES:
                        recv_ckv(j + 2, nc.scalar)  # 2 tiles in flight
                    ckv_j = ckv_js[j]
                    for m in range(HPC):
                        ps = ps2.tile([P, TPC], F32, tag="proj")
                        for kc in range(4):
                            nc.tensor.matmul(
                                ps[:], wkk_sb[:, kc * WKK + m * P: kc * WKK + (m + 1) * P],
                                ckv_j[:, kc * TPC:(kc + 1) * TPC],
                                start=(kc == 0), stop=(kc == 3))
                        nc.scalar.copy(knope[:, m * T + j * TPC: m * T + (j + 1) * TPC], ps[:])
                    for tb in range(TPC // P):
                        ps = ps2.tile([P, WKK], F32, tag="projv")
                        for kc in range(4):
                            nc.tensor.matmul(
                                ps[:], ckv_j[:, kc * TPC + tb * P: kc * TPC + (tb + 1) * P],
                                wkv_sb[:, kc * WKK:(kc + 1) * WKK],
                                start=(kc == 0), stop=(kc == 3))
                        jb = j * (TPC // P) + tb
                        nc.scalar.copy(v_sb[:, jb * WKK:(jb + 1) * WKK], ps[:])
            p2a_stack.close()
            # o_proj weight tiles; transfers are spread across attention
            # (one 512KB block per q-tile normalize) to stay under the
            # DMA-activity power brake.
            p3w_stack = ExitStack()
            p3w = p3w_stack.enter_context(
                tc.tile_pool(name="p3w", bufs=1, side="right"))
            woe_sb = p3w.tile([P, NCORES * HID], BF16)
            woo_sb = p3w.tile([P, NCORES * HID], BF16)
            oe_sb = p3w.tile([P, NCORES * TPC], BF16)
            oo_sb = p3w.tile([P, NCORES * TPC], BF16)
            part_sb = p3w.tile([P, NHID * TPC], F32)

            # ============ attention: software-pipelined, causal-trimmed ============
            with tc.tile_pool(name="att_t", bufs=1) as att_t, \
                 tc.tile_pool(name="ps_s", bufs=4, space="PSUM") as ps_s_pool, \
                 tc.tile_pool(name="ps_o", bufs=2, space="PSUM") as ps_o_pool, \
                 tc.tile_pool(name="ps_d", bufs=2, space="PSUM") as ps_d_pool:

                tasks = []
                for u in range(4):          # hl-major: (hl, bb)
                    hl, bb = u // 2, u % 2
                    for qt in range(QT_PER_B):
                        nkc = 4 * (qt + 1)
                        for kc in range(nkc):
                            tasks.append((u, hl, bb, qt, kc, kc == nkc - 1))

                state = {}  # per (u,qt): dict(ps_o, ps_d, ex tiles keyed by kc)
                ex_of = {}  # task idx -> (ex tile, m0)
                norm_pending = None

                def issue_scores(i, t):
                    u, hl, bb, qt, kc, _ = t
                    qoff = bb * S + qt * 512
                    koff = bb * S + kc * P
                    mi = kc - 4 * qt
                    m0 = mi * P if mi >= 0 else 0
                    ps_sc = ps_s_pool.tile([P, 512], F32, tag="pss")
                    nc.tensor.matmul(
                        ps_sc[:, m0:512], knope[:, hl * T + koff: hl * T + koff + P],
                        qnope[:, hl * T + qoff + m0: hl * T + qoff + 512],
                        start=True, stop=False, skip_group_check=True)
                    nc.tensor.matmul(
                        ps_sc[:, m0:512], kpe2[hl * DR: hl * DR + DR, koff: koff + P],
                        qpe[hl * DR: hl * DR + DR, qoff + m0: qoff + 512],
                        start=False, stop=True, skip_group_check=True)
                    if mi >= 0:  # additive causal mask on the diagonal block
                        nc.vector.tensor_add(ps_sc[:, m0:m0 + P],
                                             ps_sc[:, m0:m0 + P], neg_sb[:])
                    ex = att_t.tile([P, 512], BF16, tag="ex", bufs=6)
                    nc.scalar.activation(ex[:, m0:512], ps_sc[:, m0:512], AF.Exp)
                    ex_of[i] = (ex, m0)

                def issue_pv(i, t):
                    """PV + paired denominator; returns norm info if qt closed."""
                    u, hl, bb, qt, kc, is_last = t
                    ex, m0 = ex_of.pop(i)
                    key = (u, qt)
                    if key not in state:
                        state[key] = {
                            "ps_o": ps_o_pool.tile([P, 512], F32, tag="pso",
                                                   name="pso"),
                            "ps_d": ps_d_pool.tile([1, 512], F32, tag="psd",
                                                   name="psd"),
                            "pend": None,
                        }
                    st = state[key]
                    jb = bb * KB_PER_B + kc
                    nc.tensor.matmul(
                        st["ps_o"][:, m0:512],
                        v_sb[:, jb * WKK + hl * DV: jb * WKK + (hl + 1) * DV],
                        ex[:, m0:512], start=(kc == 0), stop=is_last,
                        skip_group_check=True)
                    if st["pend"] is None:
                        st["pend"] = (ex, m0)
                    else:
                        pex, pm0 = st["pend"]
                        st["pend"] = None
                        first = (kc == 1)  # pairs align with even kc
                        if pm0 < m0:  # diagonal pair: unshared leading columns
                            nc.tensor.matmul(
                                st["ps_d"][:, pm0:m0], ones_col[:], pex[:, pm0:m0],
                                start=first, stop=False, skip_group_check=True)
                        exs = att_t.tile([P, 512], BF16, tag="exs", bufs=2)
                        nc.vector.tensor_add(exs[:, m0:512], pex[:, m0:512],
                                             ex[:, m0:512])
                        nc.tensor.matmul(
                            st["ps_d"][:, m0:512], ones_col[:], exs[:, m0:512],
                            start=first, stop=is_last, skip_group_check=True)
                    if is_last:
                        recip = att_t.tile([1, 512], F32, tag="rcp", bufs=2)
                        nc.vector.reciprocal_approx_fast(recip[:], st["ps_d"][:])
                        recip_bf = att_t.tile([1, 512], BF16, tag="rcpb", bufs=2)
                        nc.scalar.copy(recip_bf[:], recip[:])
                        return (u, hl, bb, qt, st, recip_bf)
                    return None

                norm_count = [0]

                def issue_norm(info):
                    u, hl, bb, qt, st, recip = info
                    bc = ps_s_pool.tile([P, 512], F32, tag="pss", name="bc")
                    nc.tensor.matmul(bc[:], ones_row[:], recip[:], start=True,
                                     stop=True, skip_group_check=True)
                    ou = att_t.tile([P, 512], F32, tag="ou", bufs=2)
                    nc.scalar.copy(ou[:], st["ps_o"][:])
                    on = att_t.tile([P, 512], BF16, tag="on", bufs=2)
                    nc.vector.tensor_mul(on[:], ou[:], bc[:])
                    blk = bb * QT_PER_B + qt
                    tgt = oa_in if hl == 0 else ob_in
                    nc.sync.dma_start(tgt[blk * DV:(blk + 1) * DV, :], on[:])
                    del state[(u, qt)]
                    # spread o_proj weight prefetch: one 512KB block per norm
                    e = norm_count[0]
                    norm_count[0] += 1
                    if e < 8:
                        nc.sync.dma_start(woe_sb[:, e * HID:(e + 1) * HID],
                                          woT.ap()[(2 * e) * P:(2 * e + 1) * P, :])
                    else:
                        eo = e - 8
                        nc.sync.dma_start(woo_sb[:, eo * HID:(eo + 1) * HID],
                                          woT.ap()[(2 * eo + 1) * P:(2 * eo + 2) * P, :])
                    if u == 1 and qt == QT_PER_B - 1:
                        # even heads complete -> ship while odd attention runs
                        nc.gpsimd.collective_compute(
                            "AllToAll", mybir.AluOpType.bypass, replica_groups=RG,
                            ins=[oa_in.opt()], outs=[oa_out.opt()])
                        for i2 in range(NCORES):
                            nc.sync.dma_start(oe_sb[:, i2 * TPC:(i2 + 1) * TPC],
                                              oa_out[i2 * P:(i2 + 1) * P, :])

                LOOKAHEAD = 3
                pend = deque()
                for i, t in enumerate(tasks):
                    issue_scores(i, t)
                    if norm_pending is not None:
                        issue_norm(norm_pending)
                        norm_pending = None
                    if len(pend) == LOOKAHEAD:
                        pi, pt = pend.popleft()
                        norm_pending = issue_pv(pi, pt)
                    pend.append((i, t))
                while pend:
                    if norm_pending is not None:
                        issue_norm(norm_pending)
                        norm_pending = None
                    pi, pt = pend.popleft()
                    norm_pending = issue_pv(pi, pt)
                if norm_pending is not None:
                    issue_norm(norm_pending)

            nc.gpsimd.collective_compute(
                "AllToAll", mybir.AluOpType.bypass, replica_groups=RG,
                ins=[ob_in.opt()], outs=[ob_out.opt()])
            for i in range(NCORES):
                nc.sync.dma_start(oo_sb[:, i * TPC:(i + 1) * TPC],
                                  ob_out[i * P:(i + 1) * P, :])

            # ============ phase 3: o_proj (pass 1 overlaps the ob AllToAll) ============
            with tc.tile_pool(name="p3t", bufs=3) as p3t, \
                 tc.tile_pool(name="ps3", bufs=4, space="PSUM") as ps3:
                for m in range(NHID):
                    ps = ps3.tile([P, TPC], F32, tag="proj")
                    for i in range(NCORES):
                        nc.tensor.matmul(
                            ps[:], woe_sb[:, i * HID + m * P: i * HID + (m + 1) * P],
                            oe_sb[:, i * TPC:(i + 1) * TPC],
                            start=(i == 0), stop=(i == NCORES - 1))
                    nc.scalar.copy(part_sb[:, m * TPC:(m + 1) * TPC], ps[:])
                for m in range(NHID):
                    ps = ps3.tile([P, TPC], F32, tag="proj")
                    for i in range(NCORES):
                        nc.tensor.matmul(
                            ps[:], woo_sb[:, i * HID + m * P: i * HID + (m + 1) * P],
                            oo_sb[:, i * TPC:(i + 1) * TPC],
                            start=(i == 0), stop=(i == NCORES - 1))
                    ot = p3t.tile([P, TPC], F32, tag="ot")
                    nc.vector.tensor_add(ot[:], ps[:], part_sb[:, m * TPC:(m + 1) * TPC])
                    nc.sync.dma_start(outT.ap()[m * P:(m + 1) * P, :], ot[:])
            p3w_stack.close()
            att_a_stack.close()
    nc.finalize()
    return nc


def _bf16(x):
    return np.ascontiguousarray(x.astype(ml_dtypes.bfloat16))


def _rope_tables():
    inv_freq = 1.0 / (THETA ** (np.arange(0, DR, 2, dtype=np.float64) / DR))
    t = np.arange(S, dtype=np.float64)
    freqs = np.outer(t, inv_freq)
    emb = np.concatenate((freqs, freqs), axis=-1)
    return np.cos(emb).astype(np.float32), np.sin(emb).astype(np.float32)


def prepare_inputs(hidden_states, w_qa, q_a_ln_w, w_qb, w_kva, kv_a_ln_w, w_kvb, w_o):
    hidden_states = np.asarray(hidden_states, dtype=np.float32)
    w_qa = np.asarray(w_qa, dtype=np.float32)
    q_a_ln_w = np.asarray(q_a_ln_w, dtype=np.float32)
    w_qb = np.asarray(w_qb, dtype=np.float32)
    w_kva = np.asarray(w_kva, dtype=np.float32)
    kv_a_ln_w = np.asarray(kv_a_ln_w, dtype=np.float32)
    w_kvb = np.asarray(w_kvb, dtype=np.float32)
    w_o = np.asarray(w_o, dtype=np.float32)

    flat = hidden_states.reshape(T, HID)
    cos, sin = _rope_tables()          # [S, DR]
    scale = DQ ** -0.5

    pos = np.arange(T) % S
    cos_d = cos[pos].T                 # [DR, T]
    sin_d = sin[pos].T

    kp = np.arange(P)[:, None]
    qf = np.arange(512)[None, :]
    masks = _bf16(np.concatenate(
        [(qf >= kp + P * p).astype(np.float32) for p in range(4)], axis=1))

    w_qb_eff = (w_qb * q_a_ln_w[None, :]) * scale       # [H*DQ, QLR]
    w_kvb_eff = w_kvb * kv_a_ln_w[None, :]              # [H*(DN+DV), KVLR]

    # w_qb rows permuted: block A = per pair j [h0 pe | h1 pe | h0 nope],
    # block B = per pair j [h1 nope]
    rows = []
    for j in range(NCORES):
        h0, h1 = 2 * j, 2 * j + 1
        rows.append(w_qb_eff[h0 * DQ + DN: h0 * DQ + DQ])   # h0 pe (64)
        rows.append(w_qb_eff[h1 * DQ + DN: h1 * DQ + DQ])   # h1 pe (64)
        rows.append(w_qb_eff[h0 * DQ: h0 * DQ + DN])        # h0 nope (128)
    for j in range(NCORES):
        h1 = 2 * j + 1
        rows.append(w_qb_eff[h1 * DQ: h1 * DQ + DN])        # h1 nope (128)
    wqbT_full = _bf16(np.concatenate(rows, axis=0).T)       # [QLR, 3072]

    wqaT = _bf16(w_qa.T)
    wkvaT = _bf16(w_kva.T)
    woT = _bf16(w_o.T)

    in_maps = []
    for c in range(NCORES):
        heads = [HPC * c + h for h in range(HPC)]
        krows = [w_kvb_eff[h * (DN + DV): h * (DN + DV) + DN] for h in heads]
        wkvbkT_c = _bf16(np.concatenate(krows, axis=0).T)
        vrows = [w_kvb_eff[h * (DN + DV) + DN: (h + 1) * (DN + DV)] for h in heads]
        wkvbvT_c = _bf16(np.concatenate(vrows, axis=0).T)

        tok0 = c * TPC
        cosl = cos_d[:, tok0:tok0 + TPC]
        sinl = sin_d[:, tok0:tok0 + TPC]
        in_maps.append({
            "hidT": _bf16(flat[tok0:tok0 + TPC].T),
            "wqaT": wqaT, "wkvaT": wkvaT,
            "wqbT": wqbT_full, "wkvbkT": wkvbkT_c, "wkvbvT": wkvbvT_c,
            "woT": woT,
            "cosd": np.ascontiguousarray(np.concatenate([cosl, cosl], axis=0)),
            "sind": np.ascontiguousarray(np.concatenate([sinl, sinl], axis=0)),
            "masks": masks,
        })
    return in_maps


def kernel(hidden_states, w_qa, q_a_ln_w, w_qb, w_kva, kv_a_ln_w, w_kvb, w_o,
           _trace=False):
    global _NC_CACHE
    if _NC_CACHE is None:
        _NC_CACHE = build_nc()
    nc = _NC_CACHE
    in_maps = prepare_inputs(hidden_states, w_qa, q_a_ln_w, w_qb, w_kva,
                             kv_a_ln_w, w_kvb, w_o)
    res = run_bass_kernel_spmd(nc, in_maps, core_ids=list(range(NCORES)),
                               trace=_trace)
    out = np.empty((T, HID), dtype=np.float32)
    for c in range(NCORES):
        out[c * TPC:(c + 1) * TPC] = res.results[c]["outT"].T
    if _trace:
        kernel._last_result = res
    return out.reshape(B, S, HID)


# revision 24
# speedup vs baseline: 1.3238x; 1.0288x over previous
"""MLA (DeepSeek-style) attention block on 8 Trainium2 NeuronCores.

Sharding:
  phase 1 (token-parallel, 8 x 512 tokens): LoRA-A down-projections + rmsnorm
    + k_pe rope; small AllGather of the kv latents (576 dims); q up-projection
    for ALL 16 heads on the token side + rope, shipped to head shards via two
    shard-aligned AllToAlls (pe+even-nope first, odd-nope second).
  phase 2 (head-parallel, 2 heads x 2 batches per core): k/v up-projection
    from gathered kv latents; causal flash attention (k-major scores, exp on
    ScalarE, ones-matmul denominator, reciprocal + K=1 broadcast matmul for
    the divide).
  output: two AllToAlls (even heads overlap the odd-head attention; odd heads
    overlap the even half of the token-parallel output projection).

Scheduling notes (v2):
  - DMA issue order keeps the first ckv matmul start at ~1.5us (hid/wkva
    interleaved first; wqb prefetch rides the second HWDGE queue).
  - ckv and cq run kc-outer (accumulate all m-blocks per hid chunk) so
    compute starts before the weight streams finish.
  - kv up-projection runs AFTER the q up-projection so the q AllToAlls hide
    behind it.
  - attention is software-pipelined with a 2-chunk lookahead (PV/denominator
    of chunk i issue after the scores of chunk i+2) and per-q-tile
    normalization is deferred by one chunk; diagonal blocks only compute the
    causally visible column range.
  - o_proj weights prefetch during attention; output AllToAll receives issue
    right after each collective.

bf16 matmuls, fp32 PSUM accumulation + softmax statistics, fp32 output.
"""
import sys
from collections import deque
from contextlib import ExitStack

sys.path.insert(0, "/opt/trn_rl_repo")

import numpy as np
import ml_dtypes

import concourse.bacc as bacc
import concourse.mybir as mybir
import concourse.tile as tile
from concourse.bass_utils import run_bass_kernel_spmd

# ---- problem sizes (hardcoded per spec) ----
HID = 2048; H = 16; QLR = 1536; KVLR = 512
DN = 128; DR = 64; DV = 128; DQ = DN + DR
B = 2; S = 2048
THETA = 10000.0; EPS = 1e-6

NCORES = 8
T = B * S              # 4096 flattened tokens
TPC = T // NCORES      # 512 tokens per core
HPC = H // NCORES      # 2 heads per core
P = 128
NHID = HID // P        # 16
NQLR = QLR // P        # 12
CKW = KVLR + DR        # 576
QT_PER_B = S // 512    # 4 q-tiles of 512 per (b,h) unit
KB_PER_B = S // P      # 16 k-chunks of 128 per batch
WKK = HPC * DN         # 256

BF16 = mybir.dt.bfloat16
F32 = mybir.dt.float32
AF = mybir.ActivationFunctionType

_NC_CACHE = None


def _rope_dual(nc, pool, out_bf16, ps, cos_sb, sin_sb, tag):
    """RoPE on a [128, W] psum holding two 64-row head groups; writes bf16."""
    W = 512
    HDR = DR // 2
    rot = pool.tile([P, W], F32, tag=f"{tag}rot", name=f"{tag}rot")
    for g in range(2):
        o = g * DR
        nc.scalar.mul(rot[o:o + HDR, :], ps[o + HDR:o + DR, :], -1.0)
        nc.scalar.copy(rot[o + HDR:o + DR, :], ps[o:o + HDR, :])
    t1 = pool.tile([P, W], F32, tag=f"{tag}t1", name=f"{tag}t1")
    nc.vector.tensor_mul(t1[:], ps[:], cos_sb[:])
    nc.vector.tensor_mul(rot[:], rot[:], sin_sb[:])
    nc.vector.tensor_add(out_bf16[:], t1[:], rot[:])


def build_nc():
    nc = bacc.Bacc(None, target_bir_lowering=False, debug=False, num_devices=NCORES)

    # ---- per-core external inputs ----
    hidT = nc.dram_tensor("hidT", [HID, TPC], BF16, kind="ExternalInput")
    wqaT = nc.dram_tensor("wqaT", [HID, QLR], BF16, kind="ExternalInput")
    wkvaT = nc.dram_tensor("wkvaT", [HID, CKW], BF16, kind="ExternalInput")
    wqbT = nc.dram_tensor("wqbT", [QLR, H * DQ], BF16, kind="ExternalInput")
    wkvbkT = nc.dram_tensor("wkvbkT", [KVLR, HPC * DN], BF16, kind="ExternalInput")
    wkvbvT = nc.dram_tensor("wkvbvT", [KVLR, HPC * DV], BF16, kind="ExternalInput")
    woT = nc.dram_tensor("woT", [H * DV, HID], BF16, kind="ExternalInput")
    cosd = nc.dram_tensor("cosd", [P, TPC], F32, kind="ExternalInput")
    sind = nc.dram_tensor("sind", [P, TPC], F32, kind="ExternalInput")
    masks = nc.dram_tensor("masks", [P, 4 * 512], BF16, kind="ExternalInput")
    outT = nc.dram_tensor("outT", [HID, TPC], F32, kind="ExternalOutput")

    RG = [list(range(NCORES))]

    with tile.TileContext(nc) as tc:
        with tc.tile_pool(name="dram", bufs=1, space="DRAM") as dram, \
             tc.tile_pool(name="const", bufs=1) as const:
            latkv_in = dram.tile([CKW, TPC], BF16)
            latkv_all = dram.tile([NCORES * CKW, TPC], BF16, addr_space="Shared")
            qa_in = dram.tile([NCORES * 2 * P, TPC], BF16)   # [pe2|h0n] per pair
            qa_out = dram.tile([NCORES * 2 * P, TPC], BF16)
            qb_in = dram.tile([NCORES * P, TPC], BF16)       # h1n per pair
            qb_out = dram.tile([NCORES * P, TPC], BF16)
            oa_in = dram.tile([NCORES * DV, TPC], BF16)      # even heads out
            oa_out = dram.tile([NCORES * DV, TPC], BF16)
            ob_in = dram.tile([NCORES * DV, TPC], BF16)      # odd heads out
            ob_out = dram.tile([NCORES * DV, TPC], BF16)

            ones_col = const.tile([P, 1], BF16)
            nc.vector.memset(ones_col[:], 1.0)
            ones_row = const.tile([1, P], BF16)
            nc.vector.memset(ones_row[:], 1.0)
            eps_t = const.tile([1, 1], F32)
            nc.vector.memset(eps_t[:], EPS)
            cos_sb = const.tile([P, TPC], F32)
            sin_sb = const.tile([P, TPC], F32)
            mask_sb = const.tile([P, 512], BF16)
            neg_sb = const.tile([P, P], BF16)   # (mask-1)*1e30: additive causal mask

            # ============ phase-1 pools (LIFO: p1q outer, p1w innermost) ============
            p1q_stack = ExitStack()
            p1q = p1q_stack.enter_context(tc.tile_pool(name="p1q", bufs=1))
            p1t_stack = ExitStack()
            p1t = p1t_stack.enter_context(tc.tile_pool(name="p1t", bufs=1))
            ps1_stack = ExitStack()
            ps1 = ps1_stack.enter_context(tc.tile_pool(name="ps1", bufs=6, space="PSUM"))
            ps1s = ps1_stack.enter_context(tc.tile_pool(name="ps1s", bufs=1, space="PSUM"))
            ps1b = ps1_stack.enter_context(tc.tile_pool(name="ps1b", bufs=1, space="PSUM"))
            p1w_stack = ExitStack()
            p1w = p1w_stack.enter_context(tc.tile_pool(name="p1w", bufs=1))

            hid_ch = [p1w.tile([P, TPC], BF16, tag=f"hid{kc}", name=f"hid{kc}")
                      for kc in range(NHID)]
            wkva_ch = [p1w.tile([P, CKW], BF16, tag=f"wkva{kc}", name=f"wkva{kc}")
                       for kc in range(NHID)]
            wqa_ch = [p1w.tile([P, QLR], BF16, tag=f"wqa{kc}", name=f"wqa{kc}")
                      for kc in range(NHID)]
            WQB = H * DQ  # 3072
            wqb_ch = [p1q.tile([P, WQB], BF16, tag=f"wqb{kc}", name=f"wqb{kc}")
                      for kc in range(NQLR)]
            cqn_sb = p1q.tile([P, NQLR * TPC], BF16)

            # ---- DMA issue order: ALL bulk loads on the sync queue (the
            # scalar queue carries compute; HWDGE ring backpressure on it
            # stalls the norm chains) ----
            nc.sync.dma_start(hid_ch[0][:], hidT.ap()[0:P, :])
            nc.sync.dma_start(wkva_ch[0][:], wkvaT.ap()[0:P, :])
            nc.scalar.dma_start(mask_sb[:], masks.ap()[:, 0:512])
            nc.sync.dma_start(cos_sb[:], cosd.ap()[:])
            nc.sync.dma_start(sin_sb[:], sind.ap()[:])
            for kc in range(1, NHID):
                nc.sync.dma_start(hid_ch[kc][:], hidT.ap()[kc * P:(kc + 1) * P, :])
                nc.sync.dma_start(wkva_ch[kc][:], wkvaT.ap()[kc * P:(kc + 1) * P, :])
            for kc in range(NHID):
                nc.sync.dma_start(wqa_ch[kc][:], wqaT.ap()[kc * P:(kc + 1) * P, :])

            # ============ ckv joint projection (kc-outer) ============
            nc.scalar.activation(neg_sb[:], mask_sb[:, 0:P], AF.Copy,
                                 bias=-1e30, scale=1e30)

            ps_ckv = [ps1.tile([P, TPC], F32, tag="proj", name=f"psckv{m}")
                      for m in range(4)]
            ps_pe = ps1.tile([DR, TPC], F32, tag="proj")
            for kc in range(NHID):
                for m in range(4):
                    nc.tensor.matmul(ps_ckv[m][:], wkva_ch[kc][:, m * P:(m + 1) * P],
                                     hid_ch[kc][:],
                                     start=(kc == 0), stop=(kc == NHID - 1),
                                     skip_group_check=True)
                nc.tensor.matmul(ps_pe[:], wkva_ch[kc][:, KVLR:CKW], hid_ch[kc][:],
                                 start=(kc == 0), stop=(kc == NHID - 1),
                                 skip_group_check=True)

            # k_pe rope (shared across heads) -> latkv_in[512:576]
            HDR = DR // 2
            rot = p1t.tile([DR, TPC], F32, tag="rot")
            nc.scalar.mul(rot[0:HDR, :], ps_pe[HDR:DR, :], -1.0)
            nc.scalar.copy(rot[HDR:DR, :], ps_pe[0:HDR, :])
            t1 = p1t.tile([DR, TPC], F32, tag="t1")
            nc.vector.tensor_mul(t1[:], ps_pe[:], cos_sb[0:DR, :])
            nc.vector.tensor_mul(rot[:], rot[:], sin_sb[0:DR, :])
            pe_out = p1t.tile([DR, TPC], BF16, tag="peo")
            nc.vector.tensor_add(pe_out[:], t1[:], rot[:])
            nc.sync.dma_start(latkv_in[KVLR:CKW, :], pe_out[:])

            # ckv copies + squares (scalar/vector; run during cq passA below)
            ckv_bf = p1t.tile([P, 4 * TPC], BF16)
            sq_kv = [p1t.tile([P, TPC], BF16, tag="sq", name=f"sqkv{m}", bufs=6)
                     for m in range(4)]
            for m in range(4):
                nc.scalar.copy(ckv_bf[:, m * TPC:(m + 1) * TPC], ps_ckv[m][:])
                nc.vector.tensor_mul(sq_kv[m][:], ckv_bf[:, m * TPC:(m + 1) * TPC],
                                     ckv_bf[:, m * TPC:(m + 1) * TPC])
            for kc in range(NQLR):
                nc.sync.dma_start(wqb_ch[kc][:], wqbT.ap()[kc * P:(kc + 1) * P, :])

            # ============ cq pass A (m=0..5, kc-outer) ============
            def cq_block(mlist, kc):
                for mi, m in enumerate(mlist):
                    nc.tensor.matmul(ps_cq[mi][:], wqa_ch[kc][:, m * P:(m + 1) * P],
                                     hid_ch[kc][:],
                                     start=(kc == 0), stop=(kc == NHID - 1),
                                     skip_group_check=True)

            mlistA = list(range(6))
            ps_cq = [ps1.tile([P, TPC], F32, tag="proj", name=f"pscqa{m}")
                     for m in range(6)]
            cq_block(mlistA, 0)
            cq_block(mlistA, 1)
            # ssq_kv accumulation (tensor; deps on sq_kv fall in this window)
            ssq_kv = ps1s.tile([1, TPC], F32, tag="ssq")
            for m in range(4):
                nc.tensor.matmul(ssq_kv[:], ones_col[:], sq_kv[m][:],
                                 start=(m == 0), stop=(m == 3),
                                 skip_group_check=True)
            cq_block(mlistA, 2)
            # kv rmsnorm chain + broadcast
            kv_norm = p1t.tile([1, TPC], F32, tag="nrm")
            nc.scalar.activation(kv_norm[:], ssq_kv[:], AF.Sqrt, bias=eps_t[:],
                                 scale=1.0 / KVLR)
            rn_kv = p1t.tile([1, TPC], F32, tag="rn")
            nc.vector.reciprocal(rn_kv[:], kv_norm[:])
            rn_kv_bf = p1t.tile([1, TPC], BF16, tag="rnb")
            nc.scalar.copy(rn_kv_bf[:], rn_kv[:])
            bkv = ps1b.tile([P, TPC], F32, tag="bc")
            nc.tensor.matmul(bkv[:], ones_row[:], rn_kv_bf[:], start=True, stop=True,
                             skip_group_check=True)
            for m in range(4):
                lat_sb = p1t.tile([P, TPC], BF16, tag="lat", bufs=2)
                nc.vector.tensor_mul(lat_sb[:], ckv_bf[:, m * TPC:(m + 1) * TPC], bkv[:])
                nc.sync.dma_start(latkv_in[m * P:(m + 1) * P, :], lat_sb[:])
            nc.gpsimd.collective_compute(
                "AllGather", mybir.AluOpType.bypass, replica_groups=RG,
                ins=[latkv_in.opt()], outs=[latkv_all.opt()])
            for kc in range(3, NHID):
                cq_block(mlistA, kc)
            # copies + squares for pass A blocks (run during pass B)
            sq_q = {}
            for mi, m in enumerate(mlistA):
                sq_q[m] = p1t.tile([P, TPC], BF16, tag="sq", name=f"sqq{m}", bufs=6)
                nc.scalar.copy(cqn_sb[:, m * TPC:(m + 1) * TPC], ps_cq[mi][:])
                nc.vector.tensor_mul(sq_q[m][:], cqn_sb[:, m * TPC:(m + 1) * TPC],
                                     cqn_sb[:, m * TPC:(m + 1) * TPC])

            # ============ cq pass B (m=6..11, m-outer: per-block norm pipelining) ============
            mlistB = list(range(6, 12))
            ssq_q = ps1s.tile([1, TPC], F32, tag="ssq")
            for m in mlistA:
                nc.tensor.matmul(ssq_q[:], ones_col[:], sq_q[m][:],
                                 start=(m == 0), stop=False,
                                 skip_group_check=True)
            for m in mlistB:
                ps = ps1.tile([P, TPC], F32, tag="proj", name=f"pscqb{m}")
                for kc in range(NHID):
                    nc.tensor.matmul(ps[:], wqa_ch[kc][:, m * P:(m + 1) * P],
                                     hid_ch[kc][:],
                                     start=(kc == 0), stop=(kc == NHID - 1),
                                     skip_group_check=True)
                sq_q[m] = p1t.tile([P, TPC], BF16, tag="sq", name=f"sqqb{m}", bufs=6)
                nc.scalar.copy(cqn_sb[:, m * TPC:(m + 1) * TPC], ps[:])
                nc.vector.tensor_mul(sq_q[m][:], cqn_sb[:, m * TPC:(m + 1) * TPC],
                                     cqn_sb[:, m * TPC:(m + 1) * TPC])
                nc.tensor.matmul(ssq_q[:], ones_col[:], sq_q[m][:],
                                 start=False, stop=(m == 11),
                                 skip_group_check=True)
            sq_norm = p1t.tile([1, TPC], F32, tag="nrm")
            nc.scalar.activation(sq_norm[:], ssq_q[:], AF.Sqrt, bias=eps_t[:],
                                 scale=1.0 / QLR)
            rn_q = p1t.tile([1, TPC], F32, tag="rn")
            nc.vector.reciprocal(rn_q[:], sq_norm[:])
            rn_q_bf = p1t.tile([1, TPC], BF16, tag="rnb")
            nc.scalar.copy(rn_q_bf[:], rn_q[:])
            bq = ps1b.tile([P, TPC], F32, tag="bc")
            nc.tensor.matmul(bq[:], ones_row[:], rn_q_bf[:], start=True, stop=True,
                             skip_group_check=True)
            for m in range(NQLR):
                nc.vector.tensor_mul(cqn_sb[:, m * TPC:(m + 1) * TPC],
                                     cqn_sb[:, m * TPC:(m + 1) * TPC], bq[:])
            p1w_stack.close()  # free hid/wkva/wqa SBUF
            p1t_stack.close()

            # right-side pools: attention-lifetime tensors + kv-up receive
            # buffers (fresh addresses -> receives run as soon as the
            # AllGather lands, paced to 2-3 tiles in flight)
            att_a_stack = ExitStack()
            att_a = att_a_stack.enter_context(
                tc.tile_pool(name="att_a", bufs=1, side="right"))
            knope = att_a.tile([P, 2 * T], BF16)
            kpe2 = att_a.tile([P, T], BF16)    # k_pe duplicated rows
            v_sb = att_a.tile([P, (T // P) * WKK], BF16)
            qnope = att_a.tile([P, 2 * T], BF16)
            qpe = att_a.tile([P, T], BF16)     # rows 0-63 h0, 64-127 h1
            p2a_stack = ExitStack()
            p2a = p2a_stack.enter_context(
                tc.tile_pool(name="p2a", bufs=3, side="right"))
            wkk_sb = p2a.tile([P, 4 * WKK], BF16, tag="wkk", bufs=1)
            wkv_sb = p2a.tile([P, 4 * WKK], BF16, tag="wkv", bufs=1)
            for kc in range(4):
                nc.sync.dma_start(wkk_sb[:, kc * WKK:(kc + 1) * WKK],
                                  wkvbkT.ap()[kc * P:(kc + 1) * P, :])
                nc.sync.dma_start(wkv_sb[:, kc * WKK:(kc + 1) * WKK],
                                  wkvbvT.ap()[kc * P:(kc + 1) * P, :])
            ckv_js = []
            def recv_ckv(j, engine):
                basek = j * CKW
                ckv_j = p2a.tile([P, 4 * TPC], BF16, tag="ckvj", name="ckv_j",
                                 bufs=6)
                for r in range(4):
                    engine.dma_start(ckv_j[:, r * TPC:(r + 1) * TPC],
                                     latkv_all[basek + r * P: basek + (r + 1) * P, :])
                engine.dma_start(kpe2[0:DR, j * TPC:(j + 1) * TPC],
                                 latkv_all[basek + KVLR: basek + CKW, :])
                engine.dma_start(kpe2[DR:P, j * TPC:(j + 1) * TPC],
                                 latkv_all[basek + KVLR: basek + CKW, :])
                ckv_js.append(ckv_j)

            # ============ q up-projection for ALL heads ============
            with tc.tile_pool(name="p1qt", bufs=3) as p1qt:
                for mb in range(16):
                    ps = ps1.tile([P, TPC], F32, tag="proj")
                    for kc in range(NQLR):
                        nc.tensor.matmul(ps[:], wqb_ch[kc][:, mb * P:(mb + 1) * P],
                                         cqn_sb[:, kc * TPC:(kc + 1) * TPC],
                                         start=(kc == 0), stop=(kc == NQLR - 1),
                                         skip_group_check=True)
                    qo = p1qt.tile([P, TPC], BF16, tag="qo")
                    if mb % 2 == 0:  # pe2 block -> rope
                        _rope_dual(nc, p1qt, qo, ps, cos_sb, sin_sb, "q")
                    else:
                        nc.scalar.copy(qo[:], ps[:])
                    nc.sync.dma_start(qa_in[mb * P:(mb + 1) * P, :], qo[:])
                nc.gpsimd.collective_compute(
                    "AllToAll", mybir.AluOpType.bypass, replica_groups=RG,
                    ins=[qa_in.opt()], outs=[qa_out.opt()])
                for j in range(6):
                    recv_ckv(j, nc.sync)
                for mb in range(8):
                    ps = ps1.tile([P, TPC], F32, tag="proj")
                    for kc in range(NQLR):
                        nc.tensor.matmul(ps[:], wqb_ch[kc][:, (16 + mb) * P:(17 + mb) * P],
                                         cqn_sb[:, kc * TPC:(kc + 1) * TPC],
                                         start=(kc == 0), stop=(kc == NQLR - 1),
                                         skip_group_check=True)
                    qo = p1qt.tile([P, TPC], BF16, tag="qo")
                    nc.scalar.copy(qo[:], ps[:])
                    nc.sync.dma_start(qb_in[mb * P:(mb + 1) * P, :], qo[:])
                nc.gpsimd.collective_compute(
                    "AllToAll", mybir.AluOpType.bypass, replica_groups=RG,
                    ins=[qb_in.opt()], outs=[qb_out.opt()])
            ps1_stack.close()
            p1q_stack.close()

            # q receives (queue slot: after the qb stores; before paced ckv)
            for i in range(NCORES):
                nc.sync.dma_start(qpe[:, i * TPC:(i + 1) * TPC],
                                  qa_out[i * 2 * P: i * 2 * P + P, :])
                nc.sync.dma_start(qnope[:, i * TPC:(i + 1) * TPC],
                                  qa_out[i * 2 * P + P: (i + 1) * 2 * P, :])
            for i in range(NCORES):
                nc.sync.dma_start(qnope[:, T + i * TPC: T + (i + 1) * TPC],
                                  qb_out[i * P:(i + 1) * P, :])
            recv_ckv(6, nc.sync)
            recv_ckv(7, nc.sync)

            # ===== phase 2: k/v up-projection (overlaps the q AllToAlls) =====
            with tc.tile_pool(name="ps2", bufs=4, space="PSUM") as ps2:
                for j in range(NCORES):
                    ckv_j = ckv_js[j]
                    for m in range(HPC):
                        ps = ps2.tile([P, TPC], F32, tag="proj")
                        for kc in range(4):
                            nc.tensor.matmul(
                                ps[:], wkk_sb[:, kc * WKK + m * P: kc * WKK + (m + 1) * P],
                                ckv_j[:, kc * TPC:(kc + 1) * TPC],
                                start=(kc == 0), stop=(kc == 3))
                        nc.scalar.copy(knope[:, m * T + j * TPC: m * T + (j + 1) * TPC], ps[:])
                    for tb in range(TPC // P):
                        ps = ps2.tile([P, WKK], F32, tag="projv")
                        for kc in range(4):
                            nc.tensor.matmul(
                                ps[:], ckv_j[:, kc * TPC + tb * P: kc * TPC + (tb + 1) * P],
                                wkv_sb[:, kc * WKK:(kc + 1) * WKK],
                                start=(kc == 0), stop=(kc == 3))
                        jb = j * (TPC // P) + tb
                        nc.scalar.copy(v_sb[:, jb * WKK:(jb + 1) * WKK], ps[:])
            p2a_stack.close()
            # o_proj weight tiles; transfers are spread across attention
            # (one 512KB block per q-tile normalize) to stay under the
            # DMA-activity power brake.
            p3w_stack = ExitStack()
            p3w = p3w_stack.enter_context(
                tc.tile_pool(name="p3w", bufs=1, side="right"))
            woe_sb = p3w.tile([P, NCORES * HID], BF16)
            woo_sb = p3w.tile([P, NCORES * HID], BF16)
            oe_sb = p3w.tile([P, NCORES * TPC], BF16)
            oo_sb = p3w.tile([P, NCORES * TPC], BF16)
            part_sb = p3w.tile([P, NHID * TPC], F32)

            # ============ attention: software-pipelined, causal-trimmed ============
            with tc.tile_pool(name="att_t", bufs=1) as att_t, \
                 tc.tile_pool(name="ps_s", bufs=4, space="PSUM") as ps_s_pool, \
                 tc.tile_pool(name="ps_o", bufs=2, space="PSUM") as ps_o_pool, \
                 tc.tile_pool(name="ps_d", bufs=2, space="PSUM") as ps_d_pool:

                tasks = []
                for u in range(4):          # hl-major: (hl, bb)
                    hl, bb = u // 2, u % 2
                    for qt in range(QT_PER_B):
                        nkc = 4 * (qt + 1)
                        for kc in range(nkc):
                            tasks.append((u, hl, bb, qt, kc, kc == nkc - 1))

                state = {}  # per (u,qt): dict(ps_o, ps_d, ex tiles keyed by kc)
                ex_of = {}  # task idx -> (ex tile, m0)
                norm_pending = None

                def issue_scores(i, t):
                    u, hl, bb, qt, kc, _ = t
                    qoff = bb * S + qt * 512
                    koff = bb * S + kc * P
                    mi = kc - 4 * qt
                    m0 = mi * P if mi >= 0 else 0
                    ps_sc = ps_s_pool.tile([P, 512], F32, tag="pss")
                    nc.tensor.matmul(
                        ps_sc[:, m0:512], knope[:, hl * T + koff: hl * T + koff + P],
                        qnope[:, hl * T + qoff + m0: hl * T + qoff + 512],
                        start=True, stop=False, skip_group_check=True)
                    nc.tensor.matmul(
                        ps_sc[:, m0:512], kpe2[hl * DR: hl * DR + DR, koff: koff + P],
                        qpe[hl * DR: hl * DR + DR, qoff + m0: qoff + 512],
                        start=False, stop=True, skip_group_check=True)
                    if mi >= 0:  # additive causal mask on the diagonal block
                        nc.vector.tensor_add(ps_sc[:, m0:m0 + P],
                                             ps_sc[:, m0:m0 + P], neg_sb[:])
                    ex = att_t.tile([P, 512], BF16, tag="ex", bufs=6)
                    nc.scalar.activation(ex[:, m0:512], ps_sc[:, m0:512], AF.Exp)
                    ex_of[i] = (ex, m0)

                def issue_pv(i, t):
                    """PV + paired denominator; returns norm info if qt closed."""
                    u, hl, bb, qt, kc, is_last = t
                    ex, m0 = ex_of.pop(i)
                    key = (u, qt)
                    if key not in state:
                        state[key] = {
                            "ps_o": ps_o_pool.tile([P, 512], F32, tag="pso",
                                                   name="pso"),
                            "ps_d": ps_d_pool.tile([1, 512], F32, tag="psd",
                                                   name="psd"),
                            "pend": None,
                        }
                    st = state[key]
                    jb = bb * KB_PER_B + kc
                    nc.tensor.matmul(
                        st["ps_o"][:, m0:512],
                        v_sb[:, jb * WKK + hl * DV: jb * WKK + (hl + 1) * DV],
                        ex[:, m0:512], start=(kc == 0), stop=is_last,
                        skip_group_check=True)
                    if st["pend"] is None:
                        st["pend"] = (ex, m0)
                    else:
                        pex, pm0 = st["pend"]
                        st["pend"] = None
                        first = (kc == 1)  # pairs align with even kc
                        if pm0 < m0:  # diagonal pair: unshared leading columns
                            nc.tensor.matmul(
                                st["ps_d"][:, pm0:m0], ones_col[:], pex[:, pm0:m0],
                                start=first, stop=False, skip_group_check=True)
                        exs = att_t.tile([P, 512], BF16, tag="exs", bufs=2)
                        nc.vector.tensor_add(exs[:, m0:512], pex[:, m0:512],
                                             ex[:, m0:512])
                        nc.tensor.matmul(
                            st["ps_d"][:, m0:512], ones_col[:], exs[:, m0:512],
                            start=first, stop=is_last, skip_group_check=True)
                    if is_last:
                        recip = att_t.tile([1, 512], F32, tag="rcp", bufs=2)
                        nc.vector.reciprocal_approx_fast(recip[:], st["ps_d"][:])
                        recip_bf = att_t.tile([1, 512], BF16, tag="rcpb", bufs=2)
                        nc.scalar.copy(recip_bf[:], recip[:])
                        return (u, hl, bb, qt, st, recip_bf)
                    return None

                norm_count = [0]

                def issue_norm(info):
                    u, hl, bb, qt, st, recip = info
                    bc = ps_s_pool.tile([P, 512], F32, tag="pss", name="bc")
                    nc.tensor.matmul(bc[:], ones_row[:], recip[:], start=True,
                                     stop=True, skip_group_check=True)
                    ou = att_t.tile([P, 512], F32, tag="ou", bufs=2)
                    nc.scalar.copy(ou[:], st["ps_o"][:])
                    on = att_t.tile([P, 512], BF16, tag="on", bufs=2)
                    nc.vector.tensor_mul(on[:], ou[:], bc[:])
                    blk = bb * QT_PER_B + qt
                    tgt = oa_in if hl == 0 else ob_in
                    nc.sync.dma_start(tgt[blk * DV:(blk + 1) * DV, :], on[:])
                    del state[(u, qt)]
                    # spread o_proj weight prefetch across the first 14
                    # normalizes (done before the ob collective window)
                    e = norm_count[0]
                    norm_count[0] += 1
                    blocks = []
                    if e < 8:
                        blocks.append(2 * e)            # woe block e
                    if 6 <= e < 14:
                        blocks.append(2 * (e - 6) + 1)  # woo block e-6
                    for rb in blocks:
                        tgt_sb = woe_sb if rb % 2 == 0 else woo_sb
                        i2 = rb // 2
                        nc.sync.dma_start(tgt_sb[:, i2 * HID:(i2 + 1) * HID],
                                          woT.ap()[rb * P:(rb + 1) * P, :])
                    if u == 1 and qt == QT_PER_B - 1:
                        # even heads complete -> ship while odd attention runs
                        nc.gpsimd.collective_compute(
                            "AllToAll", mybir.AluOpType.bypass, replica_groups=RG,
                            ins=[oa_in.opt()], outs=[oa_out.opt()])
                        for i2 in range(NCORES):
                            nc.sync.dma_start(oe_sb[:, i2 * TPC:(i2 + 1) * TPC],
                                              oa_out[i2 * P:(i2 + 1) * P, :])

                LOOKAHEAD = 3
                pend = deque()
                for i, t in enumerate(tasks):
                    issue_scores(i, t)
                    if norm_pending is not None:
                        issue_norm(norm_pending)
                        norm_pending = None
                    if len(pend) == LOOKAHEAD:
                        pi, pt = pend.popleft()
                        norm_pending = issue_pv(pi, pt)
                    pend.append((i, t))
                while pend:
                    if norm_pending is not None:
                        issue_norm(norm_pending)
                        norm_pending = None
                    pi, pt = pend.popleft()
                    norm_pending = issue_pv(pi, pt)
                if norm_pending is not None:
                    issue_norm(norm_pending)

            nc.gpsimd.collective_compute(
                "AllToAll", mybir.AluOpType.bypass, replica_groups=RG,
                ins=[ob_in.opt()], outs=[ob_out.opt()])
            for i in range(NCORES):
                nc.sync.dma_start(oo_sb[:, i * TPC:(i + 1) * TPC],
                                  ob_out[i * P:(i + 1) * P, :])

            # ============ phase 3: o_proj (pass 1 overlaps the ob AllToAll) ============
            with tc.tile_pool(name="p3t", bufs=3) as p3t, \
                 tc.tile_pool(name="ps3", bufs=4, space="PSUM") as ps3:
                for m in range(NHID):
                    ps = ps3.tile([P, TPC], F32, tag="proj")
                    for i in range(NCORES):
                        nc.tensor.matmul(
                            ps[:], woe_sb[:, i * HID + m * P: i * HID + (m + 1) * P],
                            oe_sb[:, i * TPC:(i + 1) * TPC],
                            start=(i == 0), stop=(i == NCORES - 1))
                    nc.scalar.copy(part_sb[:, m * TPC:(m + 1) * TPC], ps[:])
                for m in range(NHID):
                    ps = ps3.tile([P, TPC], F32, tag="proj")
                    for i in range(NCORES):
                        nc.tensor.matmul(
                            ps[:], woo_sb[:, i * HID + m * P: i * HID + (m + 1) * P],
                            oo_sb[:, i * TPC:(i + 1) * TPC],
                            start=(i == 0), stop=(i == NCORES - 1))
                    ot = p3t.tile([P, TPC], F32, tag="ot")
                    nc.vector.tensor_add(ot[:], ps[:], part_sb[:, m * TPC:(m + 1) * TPC])
                    nc.sync.dma_start(outT.ap()[m * P:(m + 1) * P, :], ot[:])
            p3w_stack.close()
            att_a_stack.close()
    nc.finalize()
    return nc


def _bf16(x):
    return np.ascontiguousarray(x.astype(ml_dtypes.bfloat16))


def _rope_tables():
    inv_freq = 1.0 / (THETA ** (np.arange(0, DR, 2, dtype=np.float64) / DR))
    t = np.arange(S, dtype=np.float64)
    freqs = np.outer(t, inv_freq)
    emb = np.concatenate((freqs, freqs), axis=-1)
    return np.cos(emb).astype(np.float32), np.sin(emb).astype(np.float32)


def prepare_inputs(hidden_states, w_qa, q_a_ln_w, w_qb, w_kva, kv_a_ln_w, w_kvb, w_o):
    hidden_states = np.asarray(hidden_states, dtype=np.float32)
    w_qa = np.asarray(w_qa, dtype=np.float32)
    q_a_ln_w = np.asarray(q_a_ln_w, dtype=np.float32)
    w_qb = np.asarray(w_qb, dtype=np.float32)
    w_kva = np.asarray(w_kva, dtype=np.float32)
    kv_a_ln_w = np.asarray(kv_a_ln_w, dtype=np.float32)
    w_kvb = np.asarray(w_kvb, dtype=np.float32)
    w_o = np.asarray(w_o, dtype=np.float32)

    flat = hidden_states.reshape(T, HID)
    cos, sin = _rope_tables()          # [S, DR]
    scale = DQ ** -0.5

    pos = np.arange(T) % S
    cos_d = cos[pos].T                 # [DR, T]
    sin_d = sin[pos].T

    kp = np.arange(P)[:, None]
    qf = np.arange(512)[None, :]
    masks = _bf16(np.concatenate(
        [(qf >= kp + P * p).astype(np.float32) for p in range(4)], axis=1))

    w_qb_eff = (w_qb * q_a_ln_w[None, :]) * scale       # [H*DQ, QLR]
    w_kvb_eff = w_kvb * kv_a_ln_w[None, :]              # [H*(DN+DV), KVLR]

    # w_qb rows permuted: block A = per pair j [h0 pe | h1 pe | h0 nope],
    # block B = per pair j [h1 nope]
    rows = []
    for j in range(NCORES):
        h0, h1 = 2 * j, 2 * j + 1
        rows.append(w_qb_eff[h0 * DQ + DN: h0 * DQ + DQ])   # h0 pe (64)
        rows.append(w_qb_eff[h1 * DQ + DN: h1 * DQ + DQ])   # h1 pe (64)
        rows.append(w_qb_eff[h0 * DQ: h0 * DQ + DN])        # h0 nope (128)
    for j in range(NCORES):
        h1 = 2 * j + 1
        rows.append(w_qb_eff[h1 * DQ: h1 * DQ + DN])        # h1 nope (128)
    wqbT_full = _bf16(np.concatenate(rows, axis=0).T)       # [QLR, 3072]

    wqaT = _bf16(w_qa.T)
    wkvaT = _bf16(w_kva.T)
    woT = _bf16(w_o.T)

    in_maps = []
    for c in range(NCORES):
        heads = [HPC * c + h for h in range(HPC)]
        krows = [w_kvb_eff[h * (DN + DV): h * (DN + DV) + DN] for h in heads]
        wkvbkT_c = _bf16(np.concatenate(krows, axis=0).T)
        vrows = [w_kvb_eff[h * (DN + DV) + DN: (h + 1) * (DN + DV)] for h in heads]
        wkvbvT_c = _bf16(np.concatenate(vrows, axis=0).T)

        tok0 = c * TPC
        cosl = cos_d[:, tok0:tok0 + TPC]
        sinl = sin_d[:, tok0:tok0 + TPC]
        in_maps.append({
            "hidT": _bf16(flat[tok0:tok0 + TPC].T),
            "wqaT": wqaT, "wkvaT": wkvaT,
            "wqbT": wqbT_full, "wkvbkT": wkvbkT_c, "wkvbvT": wkvbvT_c,
            "woT": woT,
            "cosd": np.ascontiguousarray(np.concatenate([cosl, cosl], axis=0)),
            "sind": np.ascontiguousarray(np.concatenate([sinl, sinl], axis=0)),
            "masks": masks,
        })
    return in_maps


def kernel(hidden_states, w_qa, q_a_ln_w, w_qb, w_kva, kv_a_ln_w, w_kvb, w_o,
           _trace=False):
    global _NC_CACHE
    if _NC_CACHE is None:
        _NC_CACHE = build_nc()
    nc = _NC_CACHE
    in_maps = prepare_inputs(hidden_states, w_qa, q_a_ln_w, w_qb, w_kva,
                             kv_a_ln_w, w_kvb, w_o)
    res = run_bass_kernel_spmd(nc, in_maps, core_ids=list(range(NCORES)),
                               trace=_trace)
    out = np.empty((T, HID), dtype=np.float32)
    for c in range(NCORES):
        out[c * TPC:(c + 1) * TPC] = res.results[c]["outT"].T
    if _trace:
        kernel._last_result = res
    return out.reshape(B, S, HID)
